# revision 1
# baseline (speedup 1.0000x reference)
"""MFN (Memory Fusion Network) Trainium2 Bass kernel.

Strategy: data-parallel over batch (512 -> 8 cores x 64 rows). Per core the
whole T=64 recurrence runs on-chip: all matmuls on the PE in bf16 (option-B:
stationary = transposed activations [K,64], streamed = weights), fp32
elementwise, PSUM fp32 accumulation. sigmoid is computed via
sigma(x) = 0.5 + 0.5*tanh(x/2) (the 1/2 baked into weights) so the whole
kernel uses only the exp_and_others ACT table set (exp + tanh) and never pays
table switches. Cell state and h are kept doubled (D = 2c, H = 2h), with the
compensating 0.5 factors folded into the prepped weight matrices.
The tiny final head (3x [512,128] logit matmuls + global max + 2-layer MLP)
runs on the host in numpy.
"""
import numpy as np
import ml_dtypes
from contextlib import ExitStack

BF = ml_dtypes.bfloat16

# model dims (hardcoded from the problem spec)
T, NFULL, DIN = 64, 512, 556
DL, DA, DV = 300, 128, 128
HL, HA, HV = 256, 128, 128
DLP = 384                     # DL padded to 3*128
DP = DLP + DA + DV            # 640 padded input feature dim
R = 64                        # batch rows per core
NC = 8
ATT_IN = 1024
H1 = H2 = HG = 512
MEM = 256
GATES = 4 * HL + 4 * HA + 4 * HV   # 2048

F32 = None
BF16 = None


def _w_layout():
    """Column offsets of each prepped K-tiled weight inside wpack [128, W]."""
    specs = {
        # name: (K, N)
        "wg_l": (HL, 1024), "wg_a": (HA, 512), "wg_v": (HV, 512),
        "wx_l": (DLP, 1024), "wx_a": (DA, 512), "wx_v": (DV, 512),
        "a1w1": (ATT_IN, H1), "a1w2": (H1, ATT_IN),
        "a2w1": (ATT_IN, H2), "a2w2": (H2, MEM),
        "g1w1": (ATT_IN + MEM, HG), "g2w1": (ATT_IN + MEM, HG),
        "g1w2": (HG, MEM), "g2w2": (HG, MEM),
    }
    off, out = 0, {}
    for name, (k, n) in specs.items():
        kt = (k + 127) // 128
        out[name] = (off, kt, n)
        off += kt * n
    return out, off


def _b_layout():
    specs = {
        "ones": 64, "b_g": GATES, "b_a1h": H1, "b_a1o": ATT_IN,
        "b_a2h": H2, "b_a2o": MEM, "b_g1h": HG, "b_g2h": HG, "b_gw2o": 2 * MEM,
    }
    off, out = 0, {}
    for name, n in specs.items():
        out[name] = (off, n)
        off += n
    return out, off


W_LAY, W_COLS = _w_layout()
B_LAY, B_COLS = _b_layout()


def _prep_params(inp):
    """Host-side weight prep -> (wpack [128, W_COLS] bf16, bpack [1, B_COLS] bf16)."""
    f32 = np.float32

    def gate_scale_cols(w):      # w: [4h, k] torch layout -> scale i,f,o rows by 0.5
        w = w.astype(f32).copy()
        h = w.shape[0] // 4
        w[0:2 * h] *= 0.5        # i, f
        w[3 * h:4 * h] *= 0.5    # o
        return w

    wd = {}
    # LSTM weights. Whh additionally *0.5 on input (h stored doubled).
    for m, h, d, dpad in (("l", HL, DL, DLP), ("a", HA, DA, DA), ("v", HV, DV, DV)):
        whh = gate_scale_cols(inp[f"Whh_{m}"]) * 0.5          # [4h, h]
        wih = gate_scale_cols(inp[f"Wih_{m}"])                # [4h, d]
        if dpad != d:
            wih = np.concatenate([wih, np.zeros((4 * h, dpad - d), f32)], axis=1)
        wd[f"wg_{m}"] = whh.T                                  # [h, 4h]
        wd[f"wx_{m}"] = wih.T                                  # [dpad, 4h]
    wd["a1w1"] = inp["att1_w1"].astype(f32).T * 0.5            # rows: cStar doubled
    wd["a1w2"] = inp["att1_w2"].astype(f32).T
    wd["a2w1"] = inp["att2_w1"].astype(f32).T * 0.5            # attended doubled
    wd["a2w2"] = inp["att2_w2"].astype(f32).T
    for g in ("g1", "g2"):
        w1 = inp[f"{g}_w1"].astype(f32).T.copy()               # [1280, 512]
        w1[0:ATT_IN] *= 0.5                                    # attended part doubled
        wd[f"{g}w1"] = w1
        wd[f"{g}w2"] = inp[f"{g}_w2"].astype(f32).T * 0.5      # gamma tanh-trick

    wpack = np.zeros((128, W_COLS), f32)
    for name, (off, kt, n) in W_LAY.items():
        w = wd[name]
        k = w.shape[0]
        wkt = np.zeros((kt * 128, n), f32)
        wkt[:k] = w
        wpack[:, off:off + kt * n] = wkt.reshape(kt, 128, n).transpose(1, 0, 2).reshape(128, kt * n)

    def gate_scale_b(b):
        b = b.astype(f32).copy()
        h = b.shape[0] // 4
        b[0:2 * h] *= 0.5
        b[3 * h:] *= 0.5
        return b

    bd = {
        "ones": np.ones(64, f32),
        "b_g": np.concatenate([gate_scale_b(inp[f"bih_{m}"] + inp[f"bhh_{m}"])
                               for m in "lav"]),
        "b_a1h": inp["att1_b1"].astype(f32),
        "b_a1o": inp["att1_b2"].astype(f32),
        "b_a2h": inp["att2_b1"].astype(f32),
        "b_a2o": inp["att2_b2"].astype(f32),
        "b_g1h": inp["g1_b1"].astype(f32),
        "b_g2h": inp["g2_b1"].astype(f32),
        "b_gw2o": np.concatenate([inp["g1_b2"].astype(f32) * 0.5,
                                  inp["g2_b2"].astype(f32) * 0.5]),
    }
    bpack = np.zeros((1, B_COLS), f32)
    for name, (off, n) in B_LAY.items():
        bpack[0, off:off + n] = bd[name]
    return wpack.astype(BF), bpack.astype(BF)


def _prep_x(x):
    """x [T, 512, 556] -> per-core [T, 128, 5, 64] bf16 (padded, transposed)."""
    xs = []
    for c in range(NC):
        xc = x[:, c * R:(c + 1) * R, :].astype(np.float32)       # [T, 64, 556]
        xp = np.zeros((T, R, DP), np.float32)
        xp[:, :, 0:DL] = xc[:, :, 0:DL]
        xp[:, :, DLP:DLP + DA + DV] = xc[:, :, DL:]
        xt = xp.transpose(0, 2, 1)                               # [T, 640, 64]
        xs.append(np.ascontiguousarray(
            xt.reshape(T * 5, 128, R).transpose(1, 0, 2).reshape(128, T * 5 * R)).astype(BF))
    return xs


def build_nc():
    import concourse.bass as bass
    import concourse.bacc as bacc
    import concourse.tile as tile
    from concourse import mybir, masks
    global F32, BF16
    F32 = mybir.dt.float32
    BF16 = mybir.dt.bfloat16
    AF = mybir.ActivationFunctionType
    ALU = mybir.AluOpType

    nc = bacc.Bacc("TRN2", target_bir_lowering=False, debug=False, num_devices=NC)

    xt_d = nc.dram_tensor("xt", [128, T * 5 * R], BF16, kind="ExternalInput").ap()
    w_d = nc.dram_tensor("wpack", [128, W_COLS], BF16, kind="ExternalInput").ap()
    b_d = nc.dram_tensor("bpack", [1, B_COLS], BF16, kind="ExternalInput").ap()
    h_out = nc.dram_tensor("h_out", [R, 512], F32, kind="ExternalOutput").ap()
    mem_out = nc.dram_tensor("mem_out", [R, MEM], F32, kind="ExternalOutput").ap()

    def W(name):
        off, kt, n = W_LAY[name]
        return off, kt, n

    with TileBuild(nc, tile, mybir, masks, AF, ALU) as b:
        b.run(xt_d, w_d, b_d, h_out, mem_out)
    nc.compile()
    return nc


class TileBuild:
    def __init__(self, nc, tile, mybir, masks, AF, ALU):
        self.nc, self.tile, self.mybir = nc, tile, mybir
        self.masks, self.AF, self.ALU = masks, AF, ALU

    def __enter__(self):
        self.ctx = ExitStack()
        self.tc = self.ctx.enter_context(self.tile.TileContext(self.nc))
        return self

    def __exit__(self, *a):
        self.ctx.close()

    def run(self, xt_d, w_d, b_d, h_out, mem_out):
        nc, tc, ctx = self.nc, self.tc, self.ctx
        AF, ALU = self.AF, self.ALU
        f32, bf16 = F32, BF16

        const = ctx.enter_context(tc.tile_pool(name="const", bufs=1))
        wpool = ctx.enter_context(tc.tile_pool(name="wpool", bufs=1))
        state = ctx.enter_context(tc.tile_pool(name="state", bufs=1))
        xin = ctx.enter_context(tc.tile_pool(name="xin", bufs=1))
        scr = ctx.enter_context(tc.tile_pool(name="scr", bufs=2))
        stat = ctx.enter_context(tc.tile_pool(name="stat", bufs=3))
        pmm = ctx.enter_context(tc.tile_pool(name="pmm", bufs=3, space="PSUM"))
        ptp = ctx.enter_context(tc.tile_pool(name="ptp", bufs=2, space="PSUM"))

        idf = const.tile([128, 128], f32, tag="idf", name="idf")
        self.masks.make_identity(nc, idf[:])
        idb = const.tile([128, 128], bf16, tag="idb", name="idb")
        self.masks.make_identity(nc, idb[:])

        wsb = wpool.tile([128, W_COLS], bf16, tag="wsb", name="wsb")
        nc.sync.dma_start(wsb[:], w_d[:])
        bsb = wpool.tile([1, B_COLS], bf16, tag="bsb", name="bsb")
        nc.sync.dma_start(bsb[:], b_d[:])

        def wtile(name, k, cols):
            off, kt, n = W_LAY[name]
            return wsb[:, off + k * n + cols.start: off + k * n + cols.stop]

        def btile(name, cols=None):
            off, n = B_LAY[name]
            if cols is None:
                cols = slice(0, n)
            return bsb[0:1, off + cols.start: off + cols.stop]

        ones = btile("ones")

        xsb = xin.tile([128, T * 5 * R], bf16, tag="xsb", name="xsb")
        nc.sync.dma_start(xsb[:], xt_d[:])

        # persistent state
        Cd = [state.tile([R, 512], f32, tag=f"cd{i}", name=f"cd{i}") for i in range(2)]
        Mem = [state.tile([R, MEM], f32, tag=f"mem{i}", name=f"mem{i}") for i in range(2)]
        H = state.tile([R, 512], f32, tag="H", name="H")
        cT = [state.tile([128, 256], bf16, tag=f"ct{i}", name=f"ct{i}") for i in range(2)]
        hT = state.tile([128, 256], bf16, tag="hT", name="hT")
        memT = state.tile([128, 128], bf16, tag="memT", name="memT")
        for t_ in Cd + Mem + [H]:
            nc.vector.memset(t_[:], 0.0)
        for t_ in cT + [hT, memT]:
            nc.vector.memset(t_[:], 0.0)

        def preload(ps_slice, bias_ap):
            nc.tensor.matmul(ps_slice, ones, bias_ap, start=True, stop=False,
                             skip_group_check=True)

        def mm(ps_slice, lhsT, rhs, stop=False):
            nc.tensor.matmul(ps_slice, lhsT, rhs, start=False, stop=stop,
                             skip_group_check=True)

        def transpose_group(chunks, out_dtype):
            """chunks: list of ([64,128] f32/bf16 AP, ident). Returns psum tile."""
            tp = ptp.tile([128, 64 * len(chunks)], out_dtype, tag="tp", name="tp")
            for i, ap in enumerate(chunks):
                base = ap.ap_base if hasattr(ap, "ap_base") else 0
                ident = (idf if out_dtype == f32 else idb)
                nc.tensor.transpose(tp[:, i * 64:(i + 1) * 64], ap,
                                    ident[0:64, 0:64])
                # note: all our chunk APs start at partition 0 (R=64 rows)
            return tp

        for t in range(T):
            old, new = t % 2, (t + 1) % 2
            Cd_o, Cd_n = Cd[old], Cd[new]
            Mem_o, Mem_n = Mem[old], Mem[new]
            cT_o, cT_n = cT[old], cT[new]

            def xT(k):
                o = (t * 5 + k) * R
                return xsb[:, o:o + R]

            def hTl(k):
                return hT[:, k * 64:(k + 1) * 64]

            # ---- gates psums: gl [64,1024] (l), gav [64,1024] (a|v)
            gl_ps = pmm.tile([R, 1024], f32, tag="pmm", name="gl_ps")
            gav_ps = pmm.tile([R, 1024], f32, tag="pmm", name="gav_ps")
            for c in range(2):
                preload(gl_ps[:, c * 512:(c + 1) * 512],
                        btile("b_g", slice(c * 512, (c + 1) * 512)))
            preload(gav_ps[:, 0:512], btile("b_g", slice(1024, 1536)))
            preload(gav_ps[:, 512:1024], btile("b_g", slice(1536, 2048)))
            for c in range(2):
                sl = gl_ps[:, c * 512:(c + 1) * 512]
                wcols = slice(c * 512, (c + 1) * 512)
                for k in range(2):
                    mm(sl, hTl(k), wtile("wg_l", k, wcols))
                for k in range(3):
                    mm(sl, xT(k), wtile("wx_l", k, wcols), stop=(k == 2))
            mm(gav_ps[:, 0:512], hTl(2), wtile("wg_a", 0, slice(0, 512)))
            mm(gav_ps[:, 0:512], xT(3), wtile("wx_a", 0, slice(0, 512)), stop=True)
            mm(gav_ps[:, 512:1024], hTl(3), wtile("wg_v", 0, slice(0, 512)))
            mm(gav_ps[:, 512:1024], xT(4), wtile("wx_v", 0, slice(0, 512)), stop=True)

            # ---- G = tanh(gates)  (i,f,o prescaled by 0.5 in weights)
            G = scr.tile([R, GATES], f32, tag="G", name="G")
            nc.scalar.activation(G[:, 0:1024], gl_ps[:], AF.Tanh)
            nc.scalar.activation(G[:, 1024:2048], gav_ps[:], AF.Tanh)

            # ---- cell update: D_new = 0.5*(1+tf)*D_old + (1+ti)*tg
            # gate col ranges: l: i 0:256 f 256:512 g 512:768 o 768:1024
            #                  a: i 1024:1152 f .. g .. o 1408:1536 ; v: +512
            q = scr.tile([R, 512], f32, tag="q", name="q")
            p = scr.tile([R, 512], f32, tag="p", name="p")
            GR = {"l": (0, HL), "a": (1024, HA), "v": (1536, HV)}
            off_c = {"l": 0, "a": 256, "v": 384}
            for m_ in "lav":
                g0, h = GR[m_]
                c0 = off_c[m_]
                nc.vector.scalar_tensor_tensor(
                    q[:, c0:c0 + h], G[:, g0:g0 + h], 1.0,
                    G[:, g0 + 2 * h:g0 + 3 * h], ALU.add, ALU.mult)
                nc.vector.scalar_tensor_tensor(
                    p[:, c0:c0 + h], G[:, g0 + h:g0 + 2 * h], 1.0,
                    Cd_o[:, c0:c0 + h], ALU.add, ALU.mult)
            nc.vector.scalar_tensor_tensor(
                Cd_n[:], p[:], 0.5, q[:], ALU.mult, ALU.add)

            # ---- h = (1+to)*tanh(Dnew/2)  (doubled h)
            tc2 = scr.tile([R, 512], f32, tag="tc2", name="tc2")
            nc.scalar.activation(tc2[:], Cd_n[:], AF.Tanh, scale=0.5)
            for m_ in "lav":
                g0, h = GR[m_]
                c0 = off_c[m_]
                nc.vector.scalar_tensor_tensor(
                    H[:, c0:c0 + h], G[:, g0 + 3 * h:g0 + 4 * h], 1.0,
                    tc2[:, c0:c0 + h], ALU.add, ALU.mult)

            # ---- transposes: cT_new + hT (8 chunks) -> one f32 psum + 1 drain
            tp1 = ptp.tile([128, 512], f32, tag="tp", name="tp")
            for i in range(4):
                nc.tensor.transpose(tp1[:, i * 64:(i + 1) * 64],
                                    Cd_n[:, i * 128:(i + 1) * 128], idf[0:64, 0:64])
            for i in range(4):
                nc.tensor.transpose(tp1[:, 256 + i * 64:256 + (i + 1) * 64],
                                    H[:, i * 128:(i + 1) * 128], idf[0:64, 0:64])
            nc.vector.tensor_copy(cT_n[:], tp1[:, 0:256])
            nc.vector.tensor_copy(hT[:], tp1[:, 256:512])

            # ---- att1 hidden: relu(a1w1 @ cStar)
            pa1 = pmm.tile([R, 1024], f32, tag="pmm", name="pmm")
            preload(pa1[:, 0:512], btile("b_a1h"))
            for k in range(8):
                st = cT_o[:, (k % 4) * 64:(k % 4 + 1) * 64] if k < 4 \
                    else cT_n[:, (k - 4) * 64:(k - 3) * 64]
                mm(pa1[:, 0:512], st, wtile("a1w1", k, slice(0, 512)), stop=(k == 7))
            relu1 = scr.tile([R, 512], bf16, tag="relu1", name="relu1")
            nc.vector.tensor_scalar_max(relu1[:], pa1[:, 0:512], 0.0)
            tp2 = ptp.tile([128, 256], bf16, tag="tp", name="tp")
            for i in range(4):
                nc.tensor.transpose(tp2[:, i * 64:(i + 1) * 64],
                                    relu1[:, i * 128:(i + 1) * 128], idb[0:64, 0:64])
            r1T = stat.tile([128, 256], bf16, tag="r1T", name="r1T")
            nc.vector.tensor_copy(r1T[:], tp2[:])

            # ---- logits + softmax (no max-sub; exp then normalize)
            pe2 = pmm.tile([R, 1024], f32, tag="pmm", name="pmm")
            for c in range(2):
                sl = pe2[:, c * 512:(c + 1) * 512]
                preload(sl, btile("b_a1o", slice(c * 512, (c + 1) * 512)))
                for k in range(4):
                    mm(sl, r1T[:, k * 64:(k + 1) * 64],
                       wtile("a1w2", k, slice(c * 512, (c + 1) * 512)), stop=(k == 3))
            E = scr.tile([R, 1024], f32, tag="E", name="E")
            es = scr.tile([R, 1], f32, tag="es", name="es")
            nc.scalar.activation(E[:], pe2[:], AF.Exp, accum_out=es[:])
            recip = scr.tile([R, 1], f32, tag="recip", name="recip")
            nc.vector.reciprocal(recip[:], es[:])

            # ---- attended (doubled) = E * recip * Dstar
            att = scr.tile([R, 1024], bf16, tag="att", name="att")
            nc.vector.scalar_tensor_tensor(att[:, 0:512], E[:, 0:512], recip[:, 0:1],
                                           Cd_o[:], ALU.mult, ALU.mult)
            nc.vector.scalar_tensor_tensor(att[:, 512:1024], E[:, 512:1024],
                                           recip[:, 0:1], Cd_n[:], ALU.mult, ALU.mult)
            tp3 = ptp.tile([128, 512], bf16, tag="tp", name="tp")
            for i in range(8):
                nc.tensor.transpose(tp3[:, i * 64:(i + 1) * 64],
                                    att[:, i * 128:(i + 1) * 128], idb[0:64, 0:64])
            attT = stat.tile([128, 512], bf16, tag="attT", name="attT")
            nc.vector.tensor_copy(attT[:], tp3[:])

            def bothT(k):
                return attT[:, k * 64:(k + 1) * 64] if k < 8 \
                    else memT[:, (k - 8) * 64:(k - 7) * 64]

            # ---- att2 hidden + cHat
            pa2 = pmm.tile([R, 1024], f32, tag="pmm", name="pmm")
            preload(pa2[:, 0:512], btile("b_a2h"))
            for k in range(8):
                mm(pa2[:, 0:512], attT[:, k * 64:(k + 1) * 64],
                   wtile("a2w1", k, slice(0, 512)), stop=(k == 7))
            relu2 = scr.tile([R, 512], bf16, tag="relu2", name="relu2")
            nc.vector.tensor_scalar_max(relu2[:], pa2[:, 0:512], 0.0)
            tp4 = ptp.tile([128, 256], bf16, tag="tp", name="tp")
            for i in range(4):
                nc.tensor.transpose(tp4[:, i * 64:(i + 1) * 64],
                                    relu2[:, i * 128:(i + 1) * 128], idb[0:64, 0:64])
            r2T = stat.tile([128, 256], bf16, tag="r2T", name="r2T")
            nc.vector.tensor_copy(r2T[:], tp4[:])

            pc = pmm.tile([R, 1024], f32, tag="pmm", name="pmm")
            preload(pc[:, 0:256], btile("b_a2o"))
            for k in range(4):
                mm(pc[:, 0:256], r2T[:, k * 64:(k + 1) * 64],
                   wtile("a2w2", k, slice(0, 256)), stop=(k == 3))
            cHat = scr.tile([R, MEM], f32, tag="cHat", name="cHat")
            nc.scalar.activation(cHat[:], pc[:, 0:256], AF.Tanh)

            # ---- g1/g2 hidden
            pgh = pmm.tile([R, 1024], f32, tag="pmm", name="pmm")
            for gi, gn in ((0, "g1w1"), (1, "g2w1")):
                sl = pgh[:, gi * 512:(gi + 1) * 512]
                preload(sl, btile("b_g1h" if gi == 0 else "b_g2h"))
                for k in range(10):
                    mm(sl, bothT(k), wtile(gn, k, slice(0, 512)), stop=(k == 9))
            rg = scr.tile([R, 1024], bf16, tag="rg", name="rg")
            nc.vector.tensor_scalar_max(rg[:], pgh[:], 0.0)
            tp5 = ptp.tile([128, 512], bf16, tag="tp", name="tp")
            for i in range(8):
                nc.tensor.transpose(tp5[:, i * 64:(i + 1) * 64],
                                    rg[:, i * 128:(i + 1) * 128], idb[0:64, 0:64])
            rgT = stat.tile([128, 512], bf16, tag="rgT", name="rgT")
            nc.vector.tensor_copy(rgT[:], tp5[:])

            # ---- gammas (tanh-trick, 0.5 baked into g?w2 + bias)
            pga = pmm.tile([R, 1024], f32, tag="pmm", name="pmm")
            preload(pga[:, 0:512], btile("b_gw2o"))
            for k in range(4):
                mm(pga[:, 0:256], rgT[:, k * 64:(k + 1) * 64],
                   wtile("g1w2", k, slice(0, 256)), stop=(k == 3))
            for k in range(4):
                mm(pga[:, 256:512], rgT[:, 256 + k * 64:256 + (k + 1) * 64],
                   wtile("g2w2", k, slice(0, 256)), stop=(k == 3))
            Tg = scr.tile([R, 512], f32, tag="Tg", name="Tg")
            nc.scalar.activation(Tg[:], pga[:, 0:512], AF.Tanh)
            Gam = scr.tile([R, 512], f32, tag="Gam", name="Gam")
            nc.vector.tensor_scalar(Gam[:], Tg[:], 0.5, 0.5, ALU.mult, ALU.add)

            # ---- mem update
            ma = scr.tile([R, MEM], f32, tag="ma", name="ma")
            nc.vector.tensor_tensor(ma[:], Gam[:, 0:256], Mem_o[:], ALU.mult)
            mb = scr.tile([R, MEM], f32, tag="mb", name="mb")
            nc.vector.tensor_tensor(mb[:], Gam[:, 256:512], cHat[:], ALU.mult)
            nc.vector.tensor_tensor(Mem_n[:], ma[:], mb[:], ALU.add)

            tp6 = ptp.tile([128, 128], f32, tag="tp", name="tp")
            for i in range(2):
                nc.tensor.transpose(tp6[:, i * 64:(i + 1) * 64],
                                    Mem_n[:, i * 128:(i + 1) * 128], idf[0:64, 0:64])
            nc.vector.tensor_copy(memT[:], tp6[:])

        # outputs: doubled h and final mem
        nc.sync.dma_start(h_out[:], H[:])
        nc.sync.dma_start(mem_out[:], Mem[T % 2][:])


_CACHED = {}


def kernel(**inputs):
    import concourse.bass_utils as bass_utils

    x = np.asarray(inputs["x"])
    wpack, bpack = _prep_params(inputs)
    xs = _prep_x(x)

    if "nc" not in _CACHED:
        _CACHED["nc"] = build_nc()
    nc = _CACHED["nc"]

    in_maps = [{"xt": xs[c], "wpack": wpack, "bpack": bpack} for c in range(NC)]
    res = bass_utils.run_bass_kernel_spmd(nc, in_maps, list(range(NC)))

    f32 = np.float32
    hs, mems = [], []
    for c in range(NC):
        out = res.results[c]
        hs.append(out["h_out"].astype(f32) * 0.5)     # un-double
        mems.append(out["mem_out"].astype(f32))
    Hfull = np.concatenate(hs, axis=0)                # [512, 512]
    memf = np.concatenate(mems, axis=0)               # [512, 256]
    h_l, h_a, h_v = Hfull[:, 0:256], Hfull[:, 256:384], Hfull[:, 384:512]

    def ce(h, w, b):
        z = h @ np.asarray(w).astype(f32).T + np.asarray(b).astype(f32)
        return np.exp(z - z.max())

    p_l = ce(h_l, inputs["fl_w"], inputs["fl_b"])
    p_a = ce(h_a, inputs["fa_w"], inputs["fa_b"])
    p_v = ce(h_v, inputs["fv_w"], inputs["fv_b"])
    lav = p_l * np.log(p_l) + p_a * np.log(p_a) + p_v * np.log(p_v)
    last = np.concatenate([lav, memf], axis=1)
    o1 = np.maximum(last @ np.asarray(inputs["o_w1"]).astype(f32).T
                    + np.asarray(inputs["o_b1"]).astype(f32), 0)
    out = o1 @ np.asarray(inputs["o_w2"]).astype(f32).T \
        + np.asarray(inputs["o_b2"]).astype(f32)
    return out.flatten().astype(f32)



# revision 3
# speedup vs baseline: 7.2011x; 7.2011x over previous
"""MFN (Memory Fusion Network) Trainium2 Bass kernel.

Strategy: data-parallel over batch (512 -> 8 cores x 64 rows). Per core the
whole T=64 recurrence runs on-chip: all matmuls on the PE in bf16 (option-B:
stationary = transposed activations [K,64], streamed = weights), fp32
elementwise, PSUM fp32 accumulation. sigmoid is computed via
sigma(x) = 0.5 + 0.5*tanh(x/2) (the 1/2 baked into weights) so the whole
kernel uses only the exp_and_others ACT table set (exp + tanh) and never pays
table switches. Cell state and h are kept doubled (D = 2c, H = 2h), with the
compensating 0.5 factors folded into the prepped weight matrices.
The tiny final head (3x [512,128] logit matmuls + global max + 2-layer MLP)
runs on the host in numpy.

Transfer optimizations (the wall-clock under axon is dominated by shipping
inputs over the tunnel, not by device exec):
  - x is shipped as fp8 e4m3 (half the bytes) and converted to bf16 on-chip.
  - the weight+bias pack is sharded 1/8 per core and AllGather'd on-device
    (8.4 MB total over the wire instead of 67 MB replicated).
  - h/mem outputs are fused into one tensor (one D2H array).
  - the jit'd shard_map executable is built once and cached (bass_utils
    re-traces on every call).
"""
import numpy as np
import ml_dtypes
from contextlib import ExitStack

BF = ml_dtypes.bfloat16
F8 = ml_dtypes.float8_e4m3

# model dims (hardcoded from the problem spec)
T, NFULL, DIN = 64, 512, 556
DL, DA, DV = 300, 128, 128
HL, HA, HV = 256, 128, 128
DLP = 384                     # DL padded to 3*128
DP = DLP + DA + DV            # 640 padded input feature dim
R = 64                        # batch rows per core
NC = 8
ATT_IN = 1024
H1 = H2 = HG = 512
MEM = 256
GATES = 4 * HL + 4 * HA + 4 * HV   # 2048

F32 = None
BF16 = None


def _w_layout():
    """Column offsets of each prepped K-tiled weight inside wpack [128, W]."""
    specs = {
        # name: (K, N)
        "wg_l": (HL, 1024), "wg_a": (HA, 512), "wg_v": (HV, 512),
        "wx_l": (DLP, 1024), "wx_a": (DA, 512), "wx_v": (DV, 512),
        "a1w1": (ATT_IN, H1), "a1w2": (H1, ATT_IN),
        "a2w1": (ATT_IN, H2), "a2w2": (H2, MEM),
        "g1w1": (ATT_IN + MEM, HG), "g2w1": (ATT_IN + MEM, HG),
        "g1w2": (HG, MEM), "g2w2": (HG, MEM),
    }
    off, out = 0, {}
    for name, (k, n) in specs.items():
        kt = (k + 127) // 128
        out[name] = (off, kt, n)
        off += kt * n
    return out, off


def _b_layout():
    specs = {
        "ones": 64, "b_g": GATES, "b_a1h": H1, "b_a1o": ATT_IN,
        "b_a2h": H2, "b_a2o": MEM, "b_g1h": HG, "b_g2h": HG, "b_gw2o": 2 * MEM,
    }
    off, out = 0, {}
    for name, n in specs.items():
        out[name] = (off, n)
        off += n
    return out, off


W_LAY, W_COLS = _w_layout()
B_LAY, B_COLS = _b_layout()
B_CB = (B_COLS + 127) // 128          # bias block cols appended per row (47)
B_PAD = B_CB * 128                    # 6016
WSH_COLS = W_COLS + B_CB              # sharded weight row length
RSH = 128 // NC                       # 16 rows per core shard


def _prep_params(inp):
    """Host-side weight prep -> list of 8 shards [16, WSH_COLS] bf16.

    Shard c = wfull[16c:16(c+1)] where wfull [128, WSH_COLS] holds the
    K-tiled weight pack in cols [0, W_COLS) and the flattened bias pack
    scattered row-major into the trailing B_CB cols.
    """
    f32 = np.float32

    def gate_scale_cols(w):      # w: [4h, k] torch layout -> scale i,f,o rows by 0.5
        w = w.astype(f32).copy()
        h = w.shape[0] // 4
        w[0:2 * h] *= 0.5        # i, f
        w[3 * h:4 * h] *= 0.5    # o
        return w

    wd = {}
    # LSTM weights. Whh additionally *0.5 on input (h stored doubled).
    for m, h, d, dpad in (("l", HL, DL, DLP), ("a", HA, DA, DA), ("v", HV, DV, DV)):
        whh = gate_scale_cols(inp[f"Whh_{m}"]) * 0.5          # [4h, h]
        wih = gate_scale_cols(inp[f"Wih_{m}"])                # [4h, d]
        if dpad != d:
            wih = np.concatenate([wih, np.zeros((4 * h, dpad - d), f32)], axis=1)
        wd[f"wg_{m}"] = whh.T                                  # [h, 4h]
        wd[f"wx_{m}"] = wih.T                                  # [dpad, 4h]
    wd["a1w1"] = inp["att1_w1"].astype(f32).T * 0.5            # rows: cStar doubled
    wd["a1w2"] = inp["att1_w2"].astype(f32).T
    wd["a2w1"] = inp["att2_w1"].astype(f32).T * 0.5            # attended doubled
    wd["a2w2"] = inp["att2_w2"].astype(f32).T
    for g in ("g1", "g2"):
        w1 = inp[f"{g}_w1"].astype(f32).T.copy()               # [1280, 512]
        w1[0:ATT_IN] *= 0.5                                    # attended part doubled
        wd[f"{g}w1"] = w1
        wd[f"{g}w2"] = inp[f"{g}_w2"].astype(f32).T * 0.5      # gamma tanh-trick

    wfull = np.zeros((128, WSH_COLS), f32)
    for name, (off, kt, n) in W_LAY.items():
        w = wd[name]
        k = w.shape[0]
        wkt = np.zeros((kt * 128, n), f32)
        wkt[:k] = w
        wfull[:, off:off + kt * n] = wkt.reshape(kt, 128, n).transpose(1, 0, 2).reshape(128, kt * n)

    def gate_scale_b(b):
        b = b.astype(f32).copy()
        h = b.shape[0] // 4
        b[0:2 * h] *= 0.5
        b[3 * h:] *= 0.5
        return b

    bd = {
        "ones": np.ones(64, f32),
        "b_g": np.concatenate([gate_scale_b(inp[f"bih_{m}"] + inp[f"bhh_{m}"])
                               for m in "lav"]),
        "b_a1h": inp["att1_b1"].astype(f32),
        "b_a1o": inp["att1_b2"].astype(f32),
        "b_a2h": inp["att2_b1"].astype(f32),
        "b_a2o": inp["att2_b2"].astype(f32),
        "b_g1h": inp["g1_b1"].astype(f32),
        "b_g2h": inp["g2_b1"].astype(f32),
        "b_gw2o": np.concatenate([inp["g1_b2"].astype(f32) * 0.5,
                                  inp["g2_b2"].astype(f32) * 0.5]),
    }
    bflat = np.zeros(B_PAD, f32)
    for name, (off, n) in B_LAY.items():
        bflat[off:off + n] = bd[name]
    wfull[:, W_COLS:] = bflat.reshape(128, B_CB)
    wfull = wfull.astype(BF)
    return [np.ascontiguousarray(wfull[c * RSH:(c + 1) * RSH]) for c in range(NC)]


def _prep_x(x):
    """x [T, 512, 556] -> per-core [T, 128, 5, 64] fp8 (padded, transposed)."""
    xs = []
    for c in range(NC):
        xc = x[:, c * R:(c + 1) * R, :].astype(np.float32)       # [T, 64, 556]
        xp = np.zeros((T, R, DP), np.float32)
        xp[:, :, 0:DL] = xc[:, :, 0:DL]
        xp[:, :, DLP:DLP + DA + DV] = xc[:, :, DL:]
        xt = xp.transpose(0, 2, 1)                               # [T, 640, 64]
        xs.append(np.ascontiguousarray(
            xt.reshape(T * 5, 128, R).transpose(1, 0, 2).reshape(128, T * 5 * R)).astype(F8))
    return xs


def build_nc():
    import concourse.bass as bass
    import concourse.bacc as bacc
    import concourse.tile as tile
    from concourse import mybir, masks
    global F32, BF16
    F32 = mybir.dt.float32
    BF16 = mybir.dt.bfloat16
    FP8 = mybir.dt.float8e4
    AF = mybir.ActivationFunctionType
    ALU = mybir.AluOpType

    nc = bacc.Bacc("TRN2", target_bir_lowering=False, debug=False, num_devices=NC)

    xt_d = nc.dram_tensor("xt8", [128, T * 5 * R], FP8, kind="ExternalInput").ap()
    w_d = nc.dram_tensor("wsh", [RSH, WSH_COLS], BF16, kind="ExternalInput").ap()
    out_d = nc.dram_tensor("hm_out", [R, 512 + MEM], F32, kind="ExternalOutput").ap()

    with TileBuild(nc, tile, mybir, masks, AF, ALU) as b:
        b.run(xt_d, w_d, out_d, FP8)
    nc.compile()
    return nc


class TileBuild:
    def __init__(self, nc, tile, mybir, masks, AF, ALU):
        self.nc, self.tile, self.mybir = nc, tile, mybir
        self.masks, self.AF, self.ALU = masks, AF, ALU

    def __enter__(self):
        self.ctx = ExitStack()
        self.tc = self.ctx.enter_context(self.tile.TileContext(self.nc))
        return self

    def __exit__(self, *a):
        self.ctx.close()

    def run(self, xt_d, w_d, out_d, FP8):
        nc, tc, ctx = self.nc, self.tc, self.ctx
        AF, ALU = self.AF, self.ALU
        f32, bf16 = F32, BF16

        const = ctx.enter_context(tc.tile_pool(name="const", bufs=1))
        wpool = ctx.enter_context(tc.tile_pool(name="wpool", bufs=1))
        state = ctx.enter_context(tc.tile_pool(name="state", bufs=1))
        xin = ctx.enter_context(tc.tile_pool(name="xin", bufs=1))
        scr = ctx.enter_context(tc.tile_pool(name="scr", bufs=2))
        stat = ctx.enter_context(tc.tile_pool(name="stat", bufs=3))
        pmm = ctx.enter_context(tc.tile_pool(name="pmm", bufs=3, space="PSUM"))
        ptp = ctx.enter_context(tc.tile_pool(name="ptp", bufs=2, space="PSUM"))
        dram = ctx.enter_context(tc.tile_pool(name="dram", bufs=1, space="DRAM"))

        idf = const.tile([128, 128], f32, tag="idf", name="idf")
        self.masks.make_identity(nc, idf[:])
        idb = const.tile([128, 128], bf16, tag="idb", name="idb")
        self.masks.make_identity(nc, idb[:])

        # ---- weights: shard -> bounce -> AllGather -> SBUF
        win_b = dram.tile([RSH, WSH_COLS], bf16, tag="win_b", name="win_b")
        wg_b = dram.tile([128, WSH_COLS], bf16, tag="wg_b", name="wg_b")
        nc.gpsimd.dma_start(win_b[:], w_d[:])
        nc.gpsimd.collective_compute(
            "AllGather",
            self.mybir.AluOpType.bypass,
            replica_groups=[list(range(NC))],
            ins=[win_b[:].opt()],
            outs=[wg_b[:].opt()],
        )
        wsb = wpool.tile([128, W_COLS], bf16, tag="wsb", name="wsb")
        nc.sync.dma_start(wsb[:], wg_b[:, 0:W_COLS])
        bsb = wpool.tile([1, B_PAD], bf16, tag="bsb", name="bsb")
        nc.sync.dma_start(bsb[:], wg_b[:, W_COLS:WSH_COLS])

        def wtile(name, k, cols):
            off, kt, n = W_LAY[name]
            return wsb[:, off + k * n + cols.start: off + k * n + cols.stop]

        def btile(name, cols=None):
            off, n = B_LAY[name]
            if cols is None:
                cols = slice(0, n)
            return bsb[0:1, off + cols.start: off + cols.stop]

        ones = btile("ones")

        # ---- x: fp8 in, cast to bf16 by the gpsimd (software DGE) DMA
        xsb = xin.tile([128, T * 5 * R], bf16, tag="xsb", name="xsb")
        NXC = T * 5 * R
        for i in range(8):
            sl = slice(i * (NXC // 8), (i + 1) * (NXC // 8))
            nc.gpsimd.dma_start(xsb[:, sl], xt_d[:, sl])

        # persistent state
        Cd = [state.tile([R, 512], f32, tag=f"cd{i}", name=f"cd{i}") for i in range(2)]
        Mem = [state.tile([R, MEM], f32, tag=f"mem{i}", name=f"mem{i}") for i in range(2)]
        H = state.tile([R, 512], f32, tag="H", name="H")
        cT = [state.tile([128, 256], bf16, tag=f"ct{i}", name=f"ct{i}") for i in range(2)]
        hT = state.tile([128, 256], bf16, tag="hT", name="hT")
        memT = state.tile([128, 128], bf16, tag="memT", name="memT")
        for t_ in Cd + Mem + [H]:
            nc.vector.memset(t_[:], 0.0)
        for t_ in cT + [hT, memT]:
            nc.vector.memset(t_[:], 0.0)

        def preload(ps_slice, bias_ap):
            nc.tensor.matmul(ps_slice, ones, bias_ap, start=True, stop=False,
                             skip_group_check=True)

        def mm(ps_slice, lhsT, rhs, stop=False):
            nc.tensor.matmul(ps_slice, lhsT, rhs, start=False, stop=stop,
                             skip_group_check=True)

        for t in range(T):
            old, new = t % 2, (t + 1) % 2
            Cd_o, Cd_n = Cd[old], Cd[new]
            Mem_o, Mem_n = Mem[old], Mem[new]
            cT_o, cT_n = cT[old], cT[new]

            def xT(k):
                o = (t * 5 + k) * R
                return xsb[:, o:o + R]

            def hTl(k):
                return hT[:, k * 64:(k + 1) * 64]

            # ---- gates psums: gl [64,1024] (l), gav [64,1024] (a|v)
            gl_ps = pmm.tile([R, 1024], f32, tag="pmm", name="gl_ps")
            gav_ps = pmm.tile([R, 1024], f32, tag="pmm", name="gav_ps")
            for c in range(2):
                preload(gl_ps[:, c * 512:(c + 1) * 512],
                        btile("b_g", slice(c * 512, (c + 1) * 512)))
            preload(gav_ps[:, 0:512], btile("b_g", slice(1024, 1536)))
            preload(gav_ps[:, 512:1024], btile("b_g", slice(1536, 2048)))
            for c in range(2):
                sl = gl_ps[:, c * 512:(c + 1) * 512]
                wcols = slice(c * 512, (c + 1) * 512)
                for k in range(2):
                    mm(sl, hTl(k), wtile("wg_l", k, wcols))
                for k in range(3):
                    mm(sl, xT(k), wtile("wx_l", k, wcols), stop=(k == 2))
            mm(gav_ps[:, 0:512], hTl(2), wtile("wg_a", 0, slice(0, 512)))
            mm(gav_ps[:, 0:512], xT(3), wtile("wx_a", 0, slice(0, 512)), stop=True)
            mm(gav_ps[:, 512:1024], hTl(3), wtile("wg_v", 0, slice(0, 512)))
            mm(gav_ps[:, 512:1024], xT(4), wtile("wx_v", 0, slice(0, 512)), stop=True)

            # ---- G = tanh(gates)  (i,f,o prescaled by 0.5 in weights)
            G = scr.tile([R, GATES], f32, tag="G", name="G")
            nc.scalar.activation(G[:, 0:1024], gl_ps[:], AF.Tanh)
            nc.scalar.activation(G[:, 1024:2048], gav_ps[:], AF.Tanh)

            # ---- cell update: D_new = 0.5*(1+tf)*D_old + (1+ti)*tg
            # gate col ranges: l: i 0:256 f 256:512 g 512:768 o 768:1024
            #                  a: i 1024:1152 f .. g .. o 1408:1536 ; v: +512
            q = scr.tile([R, 512], f32, tag="q", name="q")
            p = scr.tile([R, 512], f32, tag="p", name="p")
            GR = {"l": (0, HL), "a": (1024, HA), "v": (1536, HV)}
            off_c = {"l": 0, "a": 256, "v": 384}
            for m_ in "lav":
                g0, h = GR[m_]
                c0 = off_c[m_]
                nc.vector.scalar_tensor_tensor(
                    q[:, c0:c0 + h], G[:, g0:g0 + h], 1.0,
                    G[:, g0 + 2 * h:g0 + 3 * h], ALU.add, ALU.mult)
                nc.vector.scalar_tensor_tensor(
                    p[:, c0:c0 + h], G[:, g0 + h:g0 + 2 * h], 1.0,
                    Cd_o[:, c0:c0 + h], ALU.add, ALU.mult)
            nc.vector.scalar_tensor_tensor(
                Cd_n[:], p[:], 0.5, q[:], ALU.mult, ALU.add)

            # ---- h = (1+to)*tanh(Dnew/2)  (doubled h)
            tc2 = scr.tile([R, 512], f32, tag="tc2", name="tc2")
            nc.scalar.activation(tc2[:], Cd_n[:], AF.Tanh, scale=0.5)
            for m_ in "lav":
                g0, h = GR[m_]
                c0 = off_c[m_]
                nc.vector.scalar_tensor_tensor(
                    H[:, c0:c0 + h], G[:, g0 + 3 * h:g0 + 4 * h], 1.0,
                    tc2[:, c0:c0 + h], ALU.add, ALU.mult)

            # ---- transposes: cT_new + hT (8 chunks) -> one f32 psum + 1 drain
            tp1 = ptp.tile([128, 512], f32, tag="tp", name="tp")
            for i in range(4):
                nc.tensor.transpose(tp1[:, i * 64:(i + 1) * 64],
                                    Cd_n[:, i * 128:(i + 1) * 128], idf[0:64, 0:64])
            for i in range(4):
                nc.tensor.transpose(tp1[:, 256 + i * 64:256 + (i + 1) * 64],
                                    H[:, i * 128:(i + 1) * 128], idf[0:64, 0:64])
            nc.vector.tensor_copy(cT_n[:], tp1[:, 0:256])
            nc.vector.tensor_copy(hT[:], tp1[:, 256:512])

            # ---- att1 hidden: relu(a1w1 @ cStar)
            pa1 = pmm.tile([R, 1024], f32, tag="pmm", name="pmm")
            preload(pa1[:, 0:512], btile("b_a1h"))
            for k in range(8):
                st = cT_o[:, (k % 4) * 64:(k % 4 + 1) * 64] if k < 4 \
                    else cT_n[:, (k - 4) * 64:(k - 3) * 64]
                mm(pa1[:, 0:512], st, wtile("a1w1", k, slice(0, 512)), stop=(k == 7))
            relu1 = scr.tile([R, 512], bf16, tag="relu1", name="relu1")
            nc.vector.tensor_scalar_max(relu1[:], pa1[:, 0:512], 0.0)
            tp2 = ptp.tile([128, 256], bf16, tag="tp", name="tp")
            for i in range(4):
                nc.tensor.transpose(tp2[:, i * 64:(i + 1) * 64],
                                    relu1[:, i * 128:(i + 1) * 128], idb[0:64, 0:64])
            r1T = stat.tile([128, 256], bf16, tag="r1T", name="r1T")
            nc.vector.tensor_copy(r1T[:], tp2[:])

            # ---- logits + softmax (no max-sub; exp then normalize)
            pe2 = pmm.tile([R, 1024], f32, tag="pmm", name="pmm")
            for c in range(2):
                sl = pe2[:, c * 512:(c + 1) * 512]
                preload(sl, btile("b_a1o", slice(c * 512, (c + 1) * 512)))
                for k in range(4):
                    mm(sl, r1T[:, k * 64:(k + 1) * 64],
                       wtile("a1w2", k, slice(c * 512, (c + 1) * 512)), stop=(k == 3))
            E = scr.tile([R, 1024], f32, tag="E", name="E")
            es = scr.tile([R, 1], f32, tag="es", name="es")
            nc.scalar.activation(E[:], pe2[:], AF.Exp, accum_out=es[:])
            recip = scr.tile([R, 1], f32, tag="recip", name="recip")
            nc.vector.reciprocal(recip[:], es[:])

            # ---- attended (doubled) = E * recip * Dstar
            att = scr.tile([R, 1024], bf16, tag="att", name="att")
            nc.vector.scalar_tensor_tensor(att[:, 0:512], E[:, 0:512], recip[:, 0:1],
                                           Cd_o[:], ALU.mult, ALU.mult)
            nc.vector.scalar_tensor_tensor(att[:, 512:1024], E[:, 512:1024],
                                           recip[:, 0:1], Cd_n[:], ALU.mult, ALU.mult)
            tp3 = ptp.tile([128, 512], bf16, tag="tp", name="tp")
            for i in range(8):
                nc.tensor.transpose(tp3[:, i * 64:(i + 1) * 64],
                                    att[:, i * 128:(i + 1) * 128], idb[0:64, 0:64])
            attT = stat.tile([128, 512], bf16, tag="attT", name="attT")
            nc.vector.tensor_copy(attT[:], tp3[:])

            def bothT(k):
                return attT[:, k * 64:(k + 1) * 64] if k < 8 \
                    else memT[:, (k - 8) * 64:(k - 7) * 64]

            # ---- att2 hidden + cHat
            pa2 = pmm.tile([R, 1024], f32, tag="pmm", name="pmm")
            preload(pa2[:, 0:512], btile("b_a2h"))
            for k in range(8):
                mm(pa2[:, 0:512], attT[:, k * 64:(k + 1) * 64],
                   wtile("a2w1", k, slice(0, 512)), stop=(k == 7))
            relu2 = scr.tile([R, 512], bf16, tag="relu2", name="relu2")
            nc.vector.tensor_scalar_max(relu2[:], pa2[:, 0:512], 0.0)
            tp4 = ptp.tile([128, 256], bf16, tag="tp", name="tp")
            for i in range(4):
                nc.tensor.transpose(tp4[:, i * 64:(i + 1) * 64],
                                    relu2[:, i * 128:(i + 1) * 128], idb[0:64, 0:64])
            r2T = stat.tile([128, 256], bf16, tag="r2T", name="r2T")
            nc.vector.tensor_copy(r2T[:], tp4[:])

            pc = pmm.tile([R, 1024], f32, tag="pmm", name="pmm")
            preload(pc[:, 0:256], btile("b_a2o"))
            for k in range(4):
                mm(pc[:, 0:256], r2T[:, k * 64:(k + 1) * 64],
                   wtile("a2w2", k, slice(0, 256)), stop=(k == 3))
            cHat = scr.tile([R, MEM], f32, tag="cHat", name="cHat")
            nc.scalar.activation(cHat[:], pc[:, 0:256], AF.Tanh)

            # ---- g1/g2 hidden
            pgh = pmm.tile([R, 1024], f32, tag="pmm", name="pmm")
            for gi, gn in ((0, "g1w1"), (1, "g2w1")):
                sl = pgh[:, gi * 512:(gi + 1) * 512]
                preload(sl, btile("b_g1h" if gi == 0 else "b_g2h"))
                for k in range(10):
                    mm(sl, bothT(k), wtile(gn, k, slice(0, 512)), stop=(k == 9))
            rg = scr.tile([R, 1024], bf16, tag="rg", name="rg")
            nc.vector.tensor_scalar_max(rg[:], pgh[:], 0.0)
            tp5 = ptp.tile([128, 512], bf16, tag="tp", name="tp")
            for i in range(8):
                nc.tensor.transpose(tp5[:, i * 64:(i + 1) * 64],
                                    rg[:, i * 128:(i + 1) * 128], idb[0:64, 0:64])
            rgT = stat.tile([128, 512], bf16, tag="rgT", name="rgT")
            nc.vector.tensor_copy(rgT[:], tp5[:])

            # ---- gammas (tanh-trick, 0.5 baked into g?w2 + bias)
            pga = pmm.tile([R, 1024], f32, tag="pmm", name="pmm")
            preload(pga[:, 0:512], btile("b_gw2o"))
            for k in range(4):
                mm(pga[:, 0:256], rgT[:, k * 64:(k + 1) * 64],
                   wtile("g1w2", k, slice(0, 256)), stop=(k == 3))
            for k in range(4):
                mm(pga[:, 256:512], rgT[:, 256 + k * 64:256 + (k + 1) * 64],
                   wtile("g2w2", k, slice(0, 256)), stop=(k == 3))
            Tg = scr.tile([R, 512], f32, tag="Tg", name="Tg")
            nc.scalar.activation(Tg[:], pga[:, 0:512], AF.Tanh)
            Gam = scr.tile([R, 512], f32, tag="Gam", name="Gam")
            nc.vector.tensor_scalar(Gam[:], Tg[:], 0.5, 0.5, ALU.mult, ALU.add)

            # ---- mem update
            ma = scr.tile([R, MEM], f32, tag="ma", name="ma")
            nc.vector.tensor_tensor(ma[:], Gam[:, 0:256], Mem_o[:], ALU.mult)
            mb = scr.tile([R, MEM], f32, tag="mb", name="mb")
            nc.vector.tensor_tensor(mb[:], Gam[:, 256:512], cHat[:], ALU.mult)
            nc.vector.tensor_tensor(Mem_n[:], ma[:], mb[:], ALU.add)

            tp6 = ptp.tile([128, 128], f32, tag="tp", name="tp")
            for i in range(2):
                nc.tensor.transpose(tp6[:, i * 64:(i + 1) * 64],
                                    Mem_n[:, i * 128:(i + 1) * 128], idf[0:64, 0:64])
            nc.vector.tensor_copy(memT[:], tp6[:])

        # outputs: doubled h and final mem, fused into one tensor
        nc.sync.dma_start(out_d[:, 0:512], H[:])
        nc.sync.dma_start(out_d[:, 512:512 + MEM], Mem[T % 2][:])


_CACHED = {}


def _build_executor(nc):
    """Build (once) a cached jit'd shard_map executable mirroring
    bass2jax.run_bass_via_pjrt, so repeat calls skip retrace/relower."""
    import jax
    from jax.sharding import Mesh, PartitionSpec
    from jax.experimental.shard_map import shard_map
    from concourse import mybir
    from concourse.bass2jax import (_bass_exec_p, install_neuronx_cc_hook,
                                    partition_id_tensor)

    install_neuronx_cc_hook()

    partition_name = nc.partition_id_tensor.name if nc.partition_id_tensor else None
    in_names, out_names, out_avals = [], [], []
    for alloc in nc.m.functions[0].allocations:
        if not isinstance(alloc, mybir.MemoryLocationSet):
            continue
        name = alloc.memorylocations[0].name
        if alloc.kind == "ExternalInput":
            if name != partition_name:
                in_names.append(name)
        elif alloc.kind == "ExternalOutput":
            out_names.append(name)
            out_avals.append(jax.core.ShapedArray(
                tuple(alloc.tensor_shape), mybir.dt.np(alloc.dtype)))
    n_params = len(in_names)
    n_outs = len(out_names)
    all_in_names = in_names + out_names + ([partition_name] if partition_name else [])

    def _body(*args):
        operands = list(args)
        if partition_name is not None:
            operands.append(partition_id_tensor())
        outs = _bass_exec_p.bind(
            *operands,
            out_avals=tuple(out_avals),
            in_names=tuple(all_in_names),
            out_names=tuple(out_names),
            lowering_input_output_aliases=(),
            sim_require_finite=True,
            sim_require_nnan=True,
            nc=nc,
        )
        return tuple(outs)

    devices = jax.devices()[:NC]
    mesh = Mesh(np.asarray(devices), ("core",))
    in_specs = (PartitionSpec("core"),) * (n_params + n_outs)
    out_specs = (PartitionSpec("core"),) * n_outs
    donate = tuple(range(n_params, n_params + n_outs))
    sharded = jax.jit(
        shard_map(_body, mesh=mesh, in_specs=in_specs, out_specs=out_specs,
                  check_rep=False),
        donate_argnums=donate, keep_unused=True)

    def execute(in_maps):
        concat_in = [np.concatenate([m[n] for m in in_maps], axis=0)
                     for n in in_names]
        zeros = [np.zeros((NC * a.shape[0], *a.shape[1:]), a.dtype)
                 for a in out_avals]
        outs = sharded(*concat_in, *zeros)
        host = [np.asarray(o) for o in outs]
        return [
            {name: host[i].reshape(NC, *out_avals[i].shape)[c]
             for i, name in enumerate(out_names)}
            for c in range(NC)
        ]

    return execute


def _get_executor():
    if "exec" not in _CACHED:
        _CACHED["nc"] = build_nc()
        _CACHED["exec"] = _build_executor(_CACHED["nc"])
    return _CACHED["exec"]


def _postprocess(res, inputs):
    """Host-side head: logits + modality fusion + output MLP (tiny)."""
    f32 = np.float32
    hs, mems = [], []
    for c in range(NC):
        out = res[c]["hm_out"].astype(f32)
        hs.append(out[:, 0:512] * 0.5)                # un-double
        mems.append(out[:, 512:512 + MEM])
    Hfull = np.concatenate(hs, axis=0)                # [512, 512]
    memf = np.concatenate(mems, axis=0)               # [512, 256]
    h_l, h_a, h_v = Hfull[:, 0:256], Hfull[:, 256:384], Hfull[:, 384:512]

    def ce(h, w, b):
        z = h @ np.asarray(w).astype(f32).T + np.asarray(b).astype(f32)
        return np.exp(z - z.max())

    p_l = ce(h_l, inputs["fl_w"], inputs["fl_b"])
    p_a = ce(h_a, inputs["fa_w"], inputs["fa_b"])
    p_v = ce(h_v, inputs["fv_w"], inputs["fv_b"])
    lav = p_l * np.log(p_l) + p_a * np.log(p_a) + p_v * np.log(p_v)
    last = np.concatenate([lav, memf], axis=1)
    o1 = np.maximum(last @ np.asarray(inputs["o_w1"]).astype(f32).T
                    + np.asarray(inputs["o_b1"]).astype(f32), 0)
    out = o1 @ np.asarray(inputs["o_w2"]).astype(f32).T \
        + np.asarray(inputs["o_b2"]).astype(f32)
    return out.flatten().astype(f32)


def kernel(**inputs):
    x = np.asarray(inputs["x"])
    wshards = _prep_params(inputs)
    xs = _prep_x(x)
    execute = _get_executor()
    in_maps = [{"xt8": xs[c], "wsh": wshards[c]} for c in range(NC)]
    res = execute(in_maps)
    return _postprocess(res, inputs)


# revision 13
# speedup vs baseline: 8.0930x; 1.1238x over previous
"""MFN (Memory Fusion Network) Trainium2 Bass kernel.

Strategy: data-parallel over batch (512 -> 8 cores x 64 rows). Per core the
whole T=64 recurrence runs on-chip: all matmuls on the PE in bf16 (option-B:
stationary = transposed activations [K,64], streamed = weights), fp32
elementwise, PSUM fp32 accumulation. sigmoid is computed via
sigma(x) = 0.5 + 0.5*tanh(x/2) (the 1/2 baked into weights) so the whole
kernel uses only the exp_and_others ACT table set (exp + tanh) and never pays
table switches. Cell state and h are kept doubled (D = 2c, H = 2h), with the
compensating 0.5 factors folded into the prepped weight matrices.
The tiny final head (3x [512,128] logit matmuls + global max + 2-layer MLP)
runs on the host in numpy.

Transfer optimizations (the wall-clock under axon is dominated by shipping
inputs over the tunnel, not by device exec):
  - x is shipped as fp8 e4m3 (half the bytes) and converted to bf16 on-chip.
  - the weight+bias pack is sharded 1/8 per core and AllGather'd on-device
    (8.4 MB total over the wire instead of 67 MB replicated).
  - h/mem outputs are fused into one tensor (one D2H array).
  - the jit'd shard_map executable is built once and cached (bass_utils
    re-traces on every call).
"""
import numpy as np
import ml_dtypes
from contextlib import ExitStack

BF = ml_dtypes.bfloat16
F8 = ml_dtypes.float8_e4m3

# model dims (hardcoded from the problem spec)
T, NFULL, DIN = 64, 512, 556
DL, DA, DV = 300, 128, 128
HL, HA, HV = 256, 128, 128
DLP = 384                     # DL padded to 3*128
DP = DLP + DA + DV            # 640 padded input feature dim
R = 64                        # batch rows per core
NC = 8
ATT_IN = 1024
H1 = H2 = HG = 512
MEM = 256
GATES = 4 * HL + 4 * HA + 4 * HV   # 2048

F32 = None
BF16 = None


def _w_layout():
    """Column offsets of each prepped K-tiled weight inside wpack [128, W]."""
    specs = {
        # name: (K, N)
        "wg_l": (HL, 1024), "wg_a": (HA, 512), "wg_v": (HV, 512),
        "wx_l": (DLP, 1024), "wx_a": (DA, 512), "wx_v": (DV, 512),
        "a1w1": (ATT_IN, H1), "a1w2": (H1, ATT_IN),
        "a2w1": (ATT_IN, H2), "a2w2": (H2, MEM),
        "g1w1": (ATT_IN + MEM, HG), "g2w1": (ATT_IN + MEM, HG),
        "g1w2": (HG, MEM), "g2w2": (HG, MEM),
        # head: logits + output MLP (run on device after the T loop)
        "wf_l": (HL, 128), "wf_a": (HA, 128), "wf_v": (HV, 128),
        "wo1": (128 + MEM, 512), "wo2": (512, 1),
    }
    off, out = 0, {}
    for name, (k, n) in specs.items():
        kt = (k + 127) // 128
        out[name] = (off, kt, n)
        off += kt * n
    return out, off


def _b_layout():
    specs = {
        "ones": 64, "b_g": GATES, "b_a1h": H1, "b_a1o": ATT_IN,
        "b_a2h": H2, "b_a2o": MEM, "b_g1h": HG, "b_g2h": HG, "b_gw2o": 2 * MEM,
    }
    off, out = 0, {}
    for name, n in specs.items():
        out[name] = (off, n)
        off += n
    return out, off


W_LAY, W_COLS = _w_layout()
B_LAY, B_COLS = _b_layout()
B_PAD = ((B_COLS + 127) // 128) * 128     # bias pack padded (6016)
B_BYTES_ROW = B_PAD * 2 // 128            # bf16 bias bytes per row (94)
WSH_COLS = W_COLS + B_BYTES_ROW           # fp8 shard row length
RSH = 128 // NC                           # 16 rows per core shard

# compact x: k-major flat layout. main block (p, k', t, r) for k' over
# k in {0,1,3,4}; k2 block (44 valid rows only) appended.
XMAIN = 128 * 4 * T * R                   # 2097152
XK2 = 44 * T * R                          # 180224
XB = XMAIN + XK2


def _prep_params(inp):
    """Host-side weight prep -> list of 8 shards [16, WSH_COLS] fp8.

    Shard c = wfull[16c:16(c+1)] where wfull [128, WSH_COLS] holds the
    K-tiled weight pack in cols [0, W_COLS) as fp8 e4m3 and the flattened
    bf16 bias pack scattered row-major (as raw bytes) into the trailing
    B_BYTES_ROW cols.
    """
    f32 = np.float32

    def gate_scale_cols(w):      # w: [4h, k] torch layout -> scale i,f,o rows by 0.5
        w = w.astype(f32).copy()
        h = w.shape[0] // 4
        w[0:2 * h] *= 0.5        # i, f
        w[3 * h:4 * h] *= 0.5    # o
        return w

    wd = {}
    # LSTM weights. Whh additionally *0.5 on input (h stored doubled).
    for m, h, d, dpad in (("l", HL, DL, DLP), ("a", HA, DA, DA), ("v", HV, DV, DV)):
        whh = gate_scale_cols(inp[f"Whh_{m}"]) * 0.5          # [4h, h]
        wih = gate_scale_cols(inp[f"Wih_{m}"])                # [4h, d]
        if dpad != d:
            wih = np.concatenate([wih, np.zeros((4 * h, dpad - d), f32)], axis=1)
        wd[f"wg_{m}"] = whh.T                                  # [h, 4h]
        wd[f"wx_{m}"] = wih.T                                  # [dpad, 4h]
    wd["a1w1"] = inp["att1_w1"].astype(f32).T * 0.5            # rows: cStar doubled
    wd["a1w2"] = inp["att1_w2"].astype(f32).T
    wd["a2w1"] = inp["att2_w1"].astype(f32).T * 0.5            # attended doubled
    wd["a2w2"] = inp["att2_w2"].astype(f32).T
    for g in ("g1", "g2"):
        w1 = inp[f"{g}_w1"].astype(f32).T.copy()               # [1280, 512]
        w1[0:ATT_IN] *= 0.5                                    # attended part doubled
        wd[f"{g}w1"] = w1
        wd[f"{g}w2"] = inp[f"{g}_w2"].astype(f32).T * 0.5      # gamma tanh-trick

    wpack = np.zeros((128, W_COLS), f32)
    for name, (off, kt, n) in W_LAY.items():
        w = wd[name]
        k = w.shape[0]
        wkt = np.zeros((kt * 128, n), f32)
        wkt[:k] = w
        wpack[:, off:off + kt * n] = wkt.reshape(kt, 128, n).transpose(1, 0, 2).reshape(128, kt * n)

    def gate_scale_b(b):
        b = b.astype(f32).copy()
        h = b.shape[0] // 4
        b[0:2 * h] *= 0.5
        b[3 * h:] *= 0.5
        return b

    bd = {
        "ones": np.ones(64, f32),
        "b_g": np.concatenate([gate_scale_b(inp[f"bih_{m}"] + inp[f"bhh_{m}"])
                               for m in "lav"]),
        "b_a1h": inp["att1_b1"].astype(f32),
        "b_a1o": inp["att1_b2"].astype(f32),
        "b_a2h": inp["att2_b1"].astype(f32),
        "b_a2o": inp["att2_b2"].astype(f32),
        "b_g1h": inp["g1_b1"].astype(f32),
        "b_g2h": inp["g2_b1"].astype(f32),
        "b_gw2o": np.concatenate([inp["g1_b2"].astype(f32) * 0.5,
                                  inp["g2_b2"].astype(f32) * 0.5]),
    }
    bflat = np.zeros(B_PAD, f32)
    for name, (off, n) in B_LAY.items():
        bflat[off:off + n] = bd[name]
    bbytes = bflat.astype(BF).view(np.uint8).reshape(128, B_BYTES_ROW)

    wfull = np.concatenate(
        [wpack.astype(F8).view(np.uint8), bbytes], axis=1).view(F8)
    return [np.ascontiguousarray(wfull[c * RSH:(c + 1) * RSH]) for c in range(NC)]


def _prep_x(x):
    """x [T, 512, 556] -> per-core flat [1, XB] fp8, k-major compact layout.

    main block: (p, k', t, r) with k' indexing k in {0,1,3,4};
    k2 block: the 44 valid rows of k-tile 2 as (p2, t, r).
    k-tile rows in the original 556 feature space:
      k0: 0:128, k1: 128:256, k2: 256:300 (x_l tail), k3: 300:428 (x_a),
      k4: 428:556 (x_v).
    """
    xs = []
    for c in range(NC):
        xc = x[:, c * R:(c + 1) * R, :].astype(np.float32)       # [T, 64, 556]
        xt = np.ascontiguousarray(xc.transpose(2, 0, 1))         # [556, T, 64]
        x8 = xt.reshape(556, T * R).astype(F8)
        main = np.concatenate(
            [x8[0:128], x8[128:256], x8[300:428], x8[428:556]], axis=0)
        buf = np.concatenate([main.reshape(-1), x8[256:300].reshape(-1)])
        xs.append(buf.reshape(1, XB))
    return xs


def build_nc():
    import concourse.bass as bass
    import concourse.bacc as bacc
    import concourse.tile as tile
    from concourse import mybir, masks
    global F32, BF16
    F32 = mybir.dt.float32
    BF16 = mybir.dt.bfloat16
    FP8 = mybir.dt.float8e4
    AF = mybir.ActivationFunctionType
    ALU = mybir.AluOpType

    nc = bacc.Bacc("TRN2", target_bir_lowering=False, debug=False, num_devices=NC)

    xt_d = nc.dram_tensor("xt8", [1, XB], FP8, kind="ExternalInput").ap()
    w_d = nc.dram_tensor("wsh", [RSH, WSH_COLS], FP8, kind="ExternalInput").ap()
    out_d = nc.dram_tensor("hm_out", [R, 512 + MEM], F32, kind="ExternalOutput").ap()

    with TileBuild(nc, tile, mybir, masks, AF, ALU) as b:
        b.run(xt_d, w_d, out_d, FP8)
    nc.compile()
    return nc


class TileBuild:
    def __init__(self, nc, tile, mybir, masks, AF, ALU):
        self.nc, self.tile, self.mybir = nc, tile, mybir
        self.masks, self.AF, self.ALU = masks, AF, ALU

    def __enter__(self):
        self.ctx = ExitStack()
        self.tc = self.ctx.enter_context(self.tile.TileContext(self.nc))
        return self

    def __exit__(self, *a):
        self.ctx.close()

    def run(self, xt_d, w_d, out_d, FP8):
        nc, tc, ctx = self.nc, self.tc, self.ctx
        AF, ALU = self.AF, self.ALU
        f32, bf16 = F32, BF16

        const = ctx.enter_context(tc.tile_pool(name="const", bufs=1))
        wpool = ctx.enter_context(tc.tile_pool(name="wpool", bufs=1))
        state = ctx.enter_context(tc.tile_pool(name="state", bufs=1))
        xin = ctx.enter_context(tc.tile_pool(name="xin", bufs=1))
        scr = ctx.enter_context(tc.tile_pool(name="scr", bufs=2))
        stat = ctx.enter_context(tc.tile_pool(name="stat", bufs=3))
        pmm = ctx.enter_context(tc.tile_pool(name="pmm", bufs=3, space="PSUM"))
        ptp = ctx.enter_context(tc.tile_pool(name="ptp", bufs=2, space="PSUM"))
        dram = ctx.enter_context(tc.tile_pool(name="dram", bufs=1, space="DRAM"))

        idf = const.tile([128, 128], f32, tag="idf", name="idf")
        self.masks.make_identity(nc, idf[:])
        idb = const.tile([128, 128], bf16, tag="idb", name="idb")
        self.masks.make_identity(nc, idb[:])

        # ---- weights: fp8 shard -> bounce -> AllGather -> SBUF (fp8 rhs)
        win_b = dram.tile([RSH, WSH_COLS], FP8, tag="win_b", name="win_b")
        wg_b = dram.tile([128, WSH_COLS], FP8, tag="wg_b", name="wg_b")
        nc.gpsimd.dma_start(win_b[:], w_d[:])
        nc.gpsimd.collective_compute(
            "AllGather",
            self.mybir.AluOpType.bypass,
            replica_groups=[list(range(NC))],
            ins=[win_b[:].opt()],
            outs=[wg_b[:].opt()],
        )
        wsb = wpool.tile([128, W_COLS], FP8, tag="wsb", name="wsb")
        nc.sync.dma_start(wsb[:], wg_b[:, 0:W_COLS])
        # bias block: bf16 bytes stashed in the trailing fp8 cols
        bsb = wpool.tile([1, B_PAD], bf16, tag="bsb", name="bsb")
        nc.sync.dma_start(bsb[:], wg_b[:, W_COLS:WSH_COLS].bitcast(bf16))

        def wtile(name, k, cols):
            off, kt, n = W_LAY[name]
            return wsb[:, off + k * n + cols.start: off + k * n + cols.stop]

        def btile(name, cols=None):
            off, n = B_LAY[name]
            if cols is None:
                cols = slice(0, n)
            return bsb[0:1, off + cols.start: off + cols.stop]

        ones = btile("ones")

        # ---- x: compact fp8 in, cast to bf16 by gpsimd DMAs. SBUF layout is
        # k-major: col = k*T*R + t*R + r.
        from concourse.ap import AP as _AP
        xsb = xin.tile([128, T * 5 * R], bf16, tag="xsb", name="xsb")
        TR = T * R
        nc.vector.memset(xsb[:, 2 * TR:3 * TR], 0.0)   # k2 pad rows 44:128
        for j, k in enumerate((0, 1, 3, 4)):
            src = _AP(xt_d.tensor, j * 128 * TR,
                      self.mybir.VecI64Pair([[TR, 128], [1, TR]]))
            nc.gpsimd.dma_start(xsb[:, k * TR:(k + 1) * TR], src)
        srck2 = _AP(xt_d.tensor, XMAIN,
                    self.mybir.VecI64Pair([[TR, 44], [1, TR]]))
        nc.gpsimd.dma_start(xsb[0:44, 2 * TR:3 * TR], srck2)

        # persistent state
        Cd = [state.tile([R, 512], f32, tag=f"cd{i}", name=f"cd{i}") for i in range(2)]
        Mem = [state.tile([R, MEM], f32, tag=f"mem{i}", name=f"mem{i}") for i in range(2)]
        H = state.tile([R, 512], f32, tag="H", name="H")
        cT = [state.tile([128, 256], bf16, tag=f"ct{i}", name=f"ct{i}") for i in range(2)]
        hT = state.tile([128, 256], bf16, tag="hT", name="hT")
        memT = state.tile([128, 128], bf16, tag="memT", name="memT")
        for t_ in Cd + Mem + [H]:
            nc.vector.memset(t_[:], 0.0)
        for t_ in cT + [hT, memT]:
            nc.vector.memset(t_[:], 0.0)

        def preload(ps_slice, bias_ap):
            nc.tensor.matmul(ps_slice, ones, bias_ap, start=True, stop=False,
                             skip_group_check=True)

        def mm(ps_slice, lhsT, rhs, stop=False):
            nc.tensor.matmul(ps_slice, lhsT, rhs, start=False, stop=stop,
                             skip_group_check=True)

        for t in range(T):
            old, new = t % 2, (t + 1) % 2
            Cd_o, Cd_n = Cd[old], Cd[new]
            Mem_o, Mem_n = Mem[old], Mem[new]
            cT_o, cT_n = cT[old], cT[new]

            def xT(k):
                o = k * T * R + t * R
                return xsb[:, o:o + R]

            def hTl(k):
                return hT[:, k * 64:(k + 1) * 64]

            # ---- gates psums: gl [64,1024] (l), gav [64,1024] (a|v)
            gl_ps = pmm.tile([R, 1024], f32, tag="pmm", name="gl_ps")
            gav_ps = pmm.tile([R, 1024], f32, tag="pmm", name="gav_ps")
            for c in range(2):
                preload(gl_ps[:, c * 512:(c + 1) * 512],
                        btile("b_g", slice(c * 512, (c + 1) * 512)))
            preload(gav_ps[:, 0:512], btile("b_g", slice(1024, 1536)))
            preload(gav_ps[:, 512:1024], btile("b_g", slice(1536, 2048)))
            for c in range(2):
                sl = gl_ps[:, c * 512:(c + 1) * 512]
                wcols = slice(c * 512, (c + 1) * 512)
                for k in range(2):
                    mm(sl, hTl(k), wtile("wg_l", k, wcols))
                for k in range(3):
                    mm(sl, xT(k), wtile("wx_l", k, wcols), stop=(k == 2))
            mm(gav_ps[:, 0:512], hTl(2), wtile("wg_a", 0, slice(0, 512)))
            mm(gav_ps[:, 0:512], xT(3), wtile("wx_a", 0, slice(0, 512)), stop=True)
            mm(gav_ps[:, 512:1024], hTl(3), wtile("wg_v", 0, slice(0, 512)))
            mm(gav_ps[:, 512:1024], xT(4), wtile("wx_v", 0, slice(0, 512)), stop=True)

            # ---- G = tanh(gates)  (i,f,o prescaled by 0.5 in weights)
            G = scr.tile([R, GATES], f32, tag="G", name="G")
            nc.scalar.activation(G[:, 0:1024], gl_ps[:], AF.Tanh)
            nc.scalar.activation(G[:, 1024:2048], gav_ps[:], AF.Tanh)

            # ---- cell update: D_new = 0.5*(1+tf)*D_old + (1+ti)*tg
            # gate col ranges: l: i 0:256 f 256:512 g 512:768 o 768:1024
            #                  a: i 1024:1152 f .. g .. o 1408:1536 ; v: +512
            q = scr.tile([R, 512], f32, tag="q", name="q")
            p = scr.tile([R, 512], f32, tag="p", name="p")
            GR = {"l": (0, HL), "a": (1024, HA), "v": (1536, HV)}
            off_c = {"l": 0, "a": 256, "v": 384}
            for m_ in "lav":
                g0, h = GR[m_]
                c0 = off_c[m_]
                nc.vector.scalar_tensor_tensor(
                    q[:, c0:c0 + h], G[:, g0:g0 + h], 1.0,
                    G[:, g0 + 2 * h:g0 + 3 * h], ALU.add, ALU.mult)
                nc.vector.scalar_tensor_tensor(
                    p[:, c0:c0 + h], G[:, g0 + h:g0 + 2 * h], 1.0,
                    Cd_o[:, c0:c0 + h], ALU.add, ALU.mult)
            nc.vector.scalar_tensor_tensor(
                Cd_n[:], p[:], 0.5, q[:], ALU.mult, ALU.add)

            # ---- h = (1+to)*tanh(Dnew/2)  (doubled h)
            tc2 = scr.tile([R, 512], f32, tag="tc2", name="tc2")
            nc.scalar.activation(tc2[:], Cd_n[:], AF.Tanh, scale=0.5)
            for m_ in "lav":
                g0, h = GR[m_]
                c0 = off_c[m_]
                nc.vector.scalar_tensor_tensor(
                    H[:, c0:c0 + h], G[:, g0 + 3 * h:g0 + 4 * h], 1.0,
                    tc2[:, c0:c0 + h], ALU.add, ALU.mult)

            # ---- transposes: cT_new + hT (8 chunks) -> one f32 psum + 1 drain
            tp1 = ptp.tile([128, 512], f32, tag="tp", name="tp")
            for i in range(4):
                nc.tensor.transpose(tp1[:, i * 64:(i + 1) * 64],
                                    Cd_n[:, i * 128:(i + 1) * 128], idf[0:64, 0:64])
            for i in range(4):
                nc.tensor.transpose(tp1[:, 256 + i * 64:256 + (i + 1) * 64],
                                    H[:, i * 128:(i + 1) * 128], idf[0:64, 0:64])
            nc.vector.tensor_copy(cT_n[:], tp1[:, 0:256])
            nc.vector.tensor_copy(hT[:], tp1[:, 256:512])

            # ---- att1 hidden: relu(a1w1 @ cStar)
            pa1 = pmm.tile([R, 1024], f32, tag="pmm", name="pmm")
            preload(pa1[:, 0:512], btile("b_a1h"))
            for k in range(8):
                st = cT_o[:, (k % 4) * 64:(k % 4 + 1) * 64] if k < 4 \
                    else cT_n[:, (k - 4) * 64:(k - 3) * 64]
                mm(pa1[:, 0:512], st, wtile("a1w1", k, slice(0, 512)), stop=(k == 7))
            relu1 = scr.tile([R, 512], bf16, tag="relu1", name="relu1")
            nc.vector.tensor_scalar_max(relu1[:], pa1[:, 0:512], 0.0)
            tp2 = ptp.tile([128, 256], bf16, tag="tp", name="tp")
            for i in range(4):
                nc.tensor.transpose(tp2[:, i * 64:(i + 1) * 64],
                                    relu1[:, i * 128:(i + 1) * 128], idb[0:64, 0:64])
            r1T = stat.tile([128, 256], bf16, tag="r1T", name="r1T")
            nc.vector.tensor_copy(r1T[:], tp2[:])

            # ---- logits + softmax (no max-sub; exp then normalize)
            pe2 = pmm.tile([R, 1024], f32, tag="pmm", name="pmm")
            for c in range(2):
                sl = pe2[:, c * 512:(c + 1) * 512]
                preload(sl, btile("b_a1o", slice(c * 512, (c + 1) * 512)))
                for k in range(4):
                    mm(sl, r1T[:, k * 64:(k + 1) * 64],
                       wtile("a1w2", k, slice(c * 512, (c + 1) * 512)), stop=(k == 3))
            E = scr.tile([R, 1024], f32, tag="E", name="E")
            es = scr.tile([R, 1], f32, tag="es", name="es")
            nc.scalar.activation(E[:], pe2[:], AF.Exp, accum_out=es[:])
            recip = scr.tile([R, 1], f32, tag="recip", name="recip")
            nc.vector.reciprocal(recip[:], es[:])

            # ---- attended (doubled) = E * recip * Dstar
            att = scr.tile([R, 1024], bf16, tag="att", name="att")
            nc.vector.scalar_tensor_tensor(att[:, 0:512], E[:, 0:512], recip[:, 0:1],
                                           Cd_o[:], ALU.mult, ALU.mult)
            nc.vector.scalar_tensor_tensor(att[:, 512:1024], E[:, 512:1024],
                                           recip[:, 0:1], Cd_n[:], ALU.mult, ALU.mult)
            tp3 = ptp.tile([128, 512], bf16, tag="tp", name="tp")
            for i in range(8):
                nc.tensor.transpose(tp3[:, i * 64:(i + 1) * 64],
                                    att[:, i * 128:(i + 1) * 128], idb[0:64, 0:64])
            attT = stat.tile([128, 512], bf16, tag="attT", name="attT")
            nc.vector.tensor_copy(attT[:], tp3[:])

            def bothT(k):
                return attT[:, k * 64:(k + 1) * 64] if k < 8 \
                    else memT[:, (k - 8) * 64:(k - 7) * 64]

            # ---- att2 hidden + cHat
            pa2 = pmm.tile([R, 1024], f32, tag="pmm", name="pmm")
            preload(pa2[:, 0:512], btile("b_a2h"))
            for k in range(8):
                mm(pa2[:, 0:512], attT[:, k * 64:(k + 1) * 64],
                   wtile("a2w1", k, slice(0, 512)), stop=(k == 7))
            relu2 = scr.tile([R, 512], bf16, tag="relu2", name="relu2")
            nc.vector.tensor_scalar_max(relu2[:], pa2[:, 0:512], 0.0)
            tp4 = ptp.tile([128, 256], bf16, tag="tp", name="tp")
            for i in range(4):
                nc.tensor.transpose(tp4[:, i * 64:(i + 1) * 64],
                                    relu2[:, i * 128:(i + 1) * 128], idb[0:64, 0:64])
            r2T = stat.tile([128, 256], bf16, tag="r2T", name="r2T")
            nc.vector.tensor_copy(r2T[:], tp4[:])

            pc = pmm.tile([R, 1024], f32, tag="pmm", name="pmm")
            preload(pc[:, 0:256], btile("b_a2o"))
            for k in range(4):
                mm(pc[:, 0:256], r2T[:, k * 64:(k + 1) * 64],
                   wtile("a2w2", k, slice(0, 256)), stop=(k == 3))
            cHat = scr.tile([R, MEM], f32, tag="cHat", name="cHat")
            nc.scalar.activation(cHat[:], pc[:, 0:256], AF.Tanh)

            # ---- g1/g2 hidden
            pgh = pmm.tile([R, 1024], f32, tag="pmm", name="pmm")
            for gi, gn in ((0, "g1w1"), (1, "g2w1")):
                sl = pgh[:, gi * 512:(gi + 1) * 512]
                preload(sl, btile("b_g1h" if gi == 0 else "b_g2h"))
                for k in range(10):
                    mm(sl, bothT(k), wtile(gn, k, slice(0, 512)), stop=(k == 9))
            rg = scr.tile([R, 1024], bf16, tag="rg", name="rg")
            nc.vector.tensor_scalar_max(rg[:], pgh[:], 0.0)
            tp5 = ptp.tile([128, 512], bf16, tag="tp", name="tp")
            for i in range(8):
                nc.tensor.transpose(tp5[:, i * 64:(i + 1) * 64],
                                    rg[:, i * 128:(i + 1) * 128], idb[0:64, 0:64])
            rgT = stat.tile([128, 512], bf16, tag="rgT", name="rgT")
            nc.vector.tensor_copy(rgT[:], tp5[:])

            # ---- gammas (tanh-trick, 0.5 baked into g?w2 + bias)
            pga = pmm.tile([R, 1024], f32, tag="pmm", name="pmm")
            preload(pga[:, 0:512], btile("b_gw2o"))
            for k in range(4):
                mm(pga[:, 0:256], rgT[:, k * 64:(k + 1) * 64],
                   wtile("g1w2", k, slice(0, 256)), stop=(k == 3))
            for k in range(4):
                mm(pga[:, 256:512], rgT[:, 256 + k * 64:256 + (k + 1) * 64],
                   wtile("g2w2", k, slice(0, 256)), stop=(k == 3))
            Tg = scr.tile([R, 512], f32, tag="Tg", name="Tg")
            nc.scalar.activation(Tg[:], pga[:, 0:512], AF.Tanh)
            Gam = scr.tile([R, 512], f32, tag="Gam", name="Gam")
            nc.vector.tensor_scalar(Gam[:], Tg[:], 0.5, 0.5, ALU.mult, ALU.add)

            # ---- mem update
            ma = scr.tile([R, MEM], f32, tag="ma", name="ma")
            nc.vector.tensor_tensor(ma[:], Gam[:, 0:256], Mem_o[:], ALU.mult)
            mb = scr.tile([R, MEM], f32, tag="mb", name="mb")
            nc.vector.tensor_tensor(mb[:], Gam[:, 256:512], cHat[:], ALU.mult)
            nc.vector.tensor_tensor(Mem_n[:], ma[:], mb[:], ALU.add)

            tp6 = ptp.tile([128, 128], f32, tag="tp", name="tp")
            for i in range(2):
                nc.tensor.transpose(tp6[:, i * 64:(i + 1) * 64],
                                    Mem_n[:, i * 128:(i + 1) * 128], idf[0:64, 0:64])
            nc.vector.tensor_copy(memT[:], tp6[:])

        # outputs: doubled h and final mem, fused into one tensor
        nc.sync.dma_start(out_d[:, 0:512], H[:])
        nc.sync.dma_start(out_d[:, 512:512 + MEM], Mem[T % 2][:])


_CACHED = {}


def _build_executor(nc):
    """Build (once) a cached jit'd shard_map executable mirroring
    bass2jax.run_bass_via_pjrt, so repeat calls skip retrace/relower."""
    import jax
    from jax.sharding import Mesh, PartitionSpec
    from jax.experimental.shard_map import shard_map
    from concourse import mybir
    from concourse.bass2jax import (_bass_exec_p, install_neuronx_cc_hook,
                                    partition_id_tensor)

    install_neuronx_cc_hook()

    partition_name = nc.partition_id_tensor.name if nc.partition_id_tensor else None
    in_names, out_names, out_avals = [], [], []
    for alloc in nc.m.functions[0].allocations:
        if not isinstance(alloc, mybir.MemoryLocationSet):
            continue
        name = alloc.memorylocations[0].name
        if alloc.kind == "ExternalInput":
            if name != partition_name:
                in_names.append(name)
        elif alloc.kind == "ExternalOutput":
            out_names.append(name)
            out_avals.append(jax.core.ShapedArray(
                tuple(alloc.tensor_shape), mybir.dt.np(alloc.dtype)))
    n_params = len(in_names)
    n_outs = len(out_names)
    all_in_names = in_names + out_names + ([partition_name] if partition_name else [])

    def _body(*args):
        operands = list(args)
        if partition_name is not None:
            operands.append(partition_id_tensor())
        outs = _bass_exec_p.bind(
            *operands,
            out_avals=tuple(out_avals),
            in_names=tuple(all_in_names),
            out_names=tuple(out_names),
            lowering_input_output_aliases=(),
            sim_require_finite=True,
            sim_require_nnan=True,
            nc=nc,
        )
        return tuple(outs)

    devices = jax.devices()[:NC]
    mesh = Mesh(np.asarray(devices), ("core",))
    in_specs = (PartitionSpec("core"),) * (n_params + n_outs)
    out_specs = (PartitionSpec("core"),) * n_outs
    donate = tuple(range(n_params, n_params + n_outs))
    sharded = jax.jit(
        shard_map(_body, mesh=mesh, in_specs=in_specs, out_specs=out_specs,
                  check_rep=False),
        donate_argnums=donate, keep_unused=True)

    def execute(in_maps):
        concat_in = [np.concatenate([m[n] for m in in_maps], axis=0)
                     for n in in_names]
        zeros = [np.zeros((NC * a.shape[0], *a.shape[1:]), a.dtype)
                 for a in out_avals]
        outs = sharded(*concat_in, *zeros)
        host = [np.asarray(o) for o in outs]
        return [
            {name: host[i].reshape(NC, *out_avals[i].shape)[c]
             for i, name in enumerate(out_names)}
            for c in range(NC)
        ]

    return execute


def _get_executor():
    if "exec" not in _CACHED:
        _CACHED["nc"] = build_nc()
        _CACHED["exec"] = _build_executor(_CACHED["nc"])
    return _CACHED["exec"]


def _postprocess(res, inputs):
    """Host-side head: logits + modality fusion + output MLP (tiny)."""
    f32 = np.float32
    hs, mems = [], []
    for c in range(NC):
        out = res[c]["hm_out"].astype(f32)
        hs.append(out[:, 0:512] * 0.5)                # un-double
        mems.append(out[:, 512:512 + MEM])
    Hfull = np.concatenate(hs, axis=0)                # [512, 512]
    memf = np.concatenate(mems, axis=0)               # [512, 256]
    h_l, h_a, h_v = Hfull[:, 0:256], Hfull[:, 256:384], Hfull[:, 384:512]

    def ce(h, w, b):
        z = h @ np.asarray(w).astype(f32).T + np.asarray(b).astype(f32)
        return np.exp(z - z.max())

    p_l = ce(h_l, inputs["fl_w"], inputs["fl_b"])
    p_a = ce(h_a, inputs["fa_w"], inputs["fa_b"])
    p_v = ce(h_v, inputs["fv_w"], inputs["fv_b"])
    lav = p_l * np.log(p_l) + p_a * np.log(p_a) + p_v * np.log(p_v)
    last = np.concatenate([lav, memf], axis=1)
    o1 = np.maximum(last @ np.asarray(inputs["o_w1"]).astype(f32).T
                    + np.asarray(inputs["o_b1"]).astype(f32), 0)
    out = o1 @ np.asarray(inputs["o_w2"]).astype(f32).T \
        + np.asarray(inputs["o_b2"]).astype(f32)
    return out.flatten().astype(f32)


def kernel(**inputs):
    x = np.asarray(inputs["x"])
    wshards = _prep_params(inputs)
    xs = _prep_x(x)
    execute = _get_executor()
    in_maps = [{"xt8": xs[c], "wsh": wshards[c]} for c in range(NC)]
    res = execute(in_maps)
    return _postprocess(res, inputs)


# revision 36
# speedup vs baseline: 10.1431x; 1.2533x over previous
"""MFN (Memory Fusion Network) Trainium2 Bass kernel.

Strategy: data-parallel over batch (512 -> 8 cores x 64 rows). Per core the
whole T=64 recurrence runs on-chip: all matmuls on the PE in bf16 (option-B:
stationary = transposed activations [K,64], streamed = weights), fp32
elementwise, PSUM fp32 accumulation. sigmoid is computed via
sigma(x) = 0.5 + 0.5*tanh(x/2) (the 1/2 baked into weights) so the whole
kernel uses only the exp_and_others ACT table set (exp + tanh) and never pays
table switches. Cell state and h are kept doubled (D = 2c, H = 2h), with the
compensating 0.5 factors folded into the prepped weight matrices.
The tiny final head (3x [512,128] logit matmuls + global max + 2-layer MLP)
runs on the host in numpy.

Transfer optimizations (the wall-clock under axon is dominated by shipping
inputs over the tunnel, not by device exec):
  - x is shipped as fp8 e4m3 (half the bytes) and converted to bf16 on-chip.
  - the weight+bias pack is sharded 1/8 per core and AllGather'd on-device
    (8.4 MB total over the wire instead of 67 MB replicated).
  - h/mem outputs are fused into one tensor (one D2H array).
  - the jit'd shard_map executable is built once and cached (bass_utils
    re-traces on every call).
"""
import numpy as np
import ml_dtypes
from contextlib import ExitStack

BF = ml_dtypes.bfloat16
DEBUG_HEAD = False
F8 = ml_dtypes.float8_e4m3

# model dims (hardcoded from the problem spec)
T, NFULL, DIN = 64, 512, 556
DL, DA, DV = 300, 128, 128
HL, HA, HV = 256, 128, 128
DLP = 384                     # DL padded to 3*128
DP = DLP + DA + DV            # 640 padded input feature dim
R = 64                        # batch rows per core
NC = 8
ATT_IN = 1024
H1 = H2 = HG = 512
MEM = 256
GATES = 4 * HL + 4 * HA + 4 * HV   # 2048

F32 = None
BF16 = None


def _w_layout():
    """Column offsets of each prepped K-tiled weight inside wpack [128, W]."""
    specs = {
        # name: (K, N)
        "wg_l": (HL, 1024), "wg_a": (HA, 512), "wg_v": (HV, 512),
        "wx_l": (DLP, 1024), "wx_a": (DA, 512), "wx_v": (DV, 512),
        "a1w1": (ATT_IN, H1), "a1w2": (H1, ATT_IN),
        "a2w1": (ATT_IN, H2), "a2w2": (H2, MEM),
        "g1w1": (ATT_IN + MEM, HG), "g2w1": (ATT_IN + MEM, HG),
        "g1w2": (HG, MEM), "g2w2": (HG, MEM),
    }
    off, out = 0, {}
    for name, (k, n) in specs.items():
        kt = (k + 127) // 128
        out[name] = (off, kt, n)
        off += kt * n
    return out, off


def _h_layout():
    """Head weights stay bf16 (final dot has heavy cancellation; fp8 fails)."""
    H_OUT = 256
    specs = {
        "wf_l": (HL, 128), "wf_a": (HA, 128), "wf_v": (HV, 128),
        "wo1": (128 + MEM, H_OUT), "wo2": (H_OUT, 1),
    }
    off, out = 0, {}
    for name, (k, n) in specs.items():
        kt = (k + 127) // 128
        out[name] = (off, kt, n)
        off += kt * n
    return out, off


def _b_layout():
    specs = {
        "ones": 64, "b_g": GATES, "b_a1h": H1, "b_a1o": ATT_IN,
        "b_a2h": H2, "b_a2o": MEM, "b_g1h": HG, "b_g2h": HG, "b_gw2o": 2 * MEM,
        "b_f": 3 * 128, "b_o1": 256, "b_o2": 1,
    }
    off, out = 0, {}
    for name, n in specs.items():
        out[name] = (off, n)
        off += n
    return out, off


W_LAY, W_COLS = _w_layout()
H_LAY, H_COLS = _h_layout()
B_LAY, B_COLS = _b_layout()
B_PAD = ((B_COLS + 127) // 128) * 128     # bias pack padded
B_ROW = B_PAD // 128                      # bias bf16 elements per row
WHB_COLS = H_COLS + B_ROW                 # bf16 head+bias shard row length
RSH = 128 // NC                           # 16 rows per core shard

# compact x: k-major flat layout. main block (p, k', t, r) for k' over
# k in {0,1,3,4}; k2 block (44 valid rows only) appended.
XMAIN = 128 * 4 * T * R                   # 2097152
XK2 = 44 * T * R                          # 180224
XB = XMAIN + XK2


def _prep_params(inp):
    """Host-side weight prep -> (w8 [128, W_COLS] fp8, whb [128, WHB_COLS]
    bf16). Row-sharded 1/8 per core by shard_map; AllGather'd on device.
    Head weights + biases travel as genuine bf16 (a byte-packed fp8 ride
    corrupts them: the transport canonicalizes fp8-NaN byte patterns)."""
    f32 = np.float32

    def gate_scale_cols(w):      # w: [4h, k] torch layout -> scale i,f,o rows by 0.5
        w = w.astype(f32).copy()
        h = w.shape[0] // 4
        w[0:2 * h] *= 0.5        # i, f
        w[3 * h:4 * h] *= 0.5    # o
        return w

    wd = {}
    # LSTM weights. Whh additionally *0.5 on input (h stored doubled).
    for m, h, d, dpad in (("l", HL, DL, DLP), ("a", HA, DA, DA), ("v", HV, DV, DV)):
        whh = gate_scale_cols(inp[f"Whh_{m}"]) * 0.5          # [4h, h]
        wih = gate_scale_cols(inp[f"Wih_{m}"])                # [4h, d]
        if dpad != d:
            wih = np.concatenate([wih, np.zeros((4 * h, dpad - d), f32)], axis=1)
        wd[f"wg_{m}"] = whh.T                                  # [h, 4h]
        wd[f"wx_{m}"] = wih.T                                  # [dpad, 4h]
    wd["a1w1"] = inp["att1_w1"].astype(f32).T * 0.5            # rows: cStar doubled
    wd["a1w2"] = inp["att1_w2"].astype(f32).T
    wd["a2w1"] = inp["att2_w1"].astype(f32).T * 0.5            # attended doubled
    wd["a2w2"] = inp["att2_w2"].astype(f32).T
    for g in ("g1", "g2"):
        w1 = inp[f"{g}_w1"].astype(f32).T.copy()               # [1280, 512]
        w1[0:ATT_IN] *= 0.5                                    # attended part doubled
        wd[f"{g}w1"] = w1
        wd[f"{g}w2"] = inp[f"{g}_w2"].astype(f32).T * 0.5      # gamma tanh-trick

    hd = {
        # logits weights; 0.5 folded (h stored doubled)
        "wf_l": inp["fl_w"].astype(f32).T * 0.5,
        "wf_a": inp["fa_w"].astype(f32).T * 0.5,
        "wf_v": inp["fv_w"].astype(f32).T * 0.5,
        "wo1": inp["o_w1"].astype(f32).T,                      # [384, 512]
        "wo2": inp["o_w2"].astype(f32).T,                      # [512, 1]
    }

    def pack(lay, cols, src):
        out = np.zeros((128, cols), f32)
        for name, (off, kt, n) in lay.items():
            w = src[name]
            k = w.shape[0]
            wkt = np.zeros((kt * 128, n), f32)
            wkt[:k] = w
            out[:, off:off + kt * n] = \
                wkt.reshape(kt, 128, n).transpose(1, 0, 2).reshape(128, kt * n)
        return out

    wpack = pack(W_LAY, W_COLS, wd)
    hpack = pack(H_LAY, H_COLS, hd)

    def gate_scale_b(b):
        b = b.astype(f32).copy()
        h = b.shape[0] // 4
        b[0:2 * h] *= 0.5
        b[3 * h:] *= 0.5
        return b

    bd = {
        "ones": np.ones(64, f32),
        "b_g": np.concatenate([gate_scale_b(inp[f"bih_{m}"] + inp[f"bhh_{m}"])
                               for m in "lav"]),
        "b_a1h": inp["att1_b1"].astype(f32),
        "b_a1o": inp["att1_b2"].astype(f32),
        "b_a2h": inp["att2_b1"].astype(f32),
        "b_a2o": inp["att2_b2"].astype(f32),
        "b_g1h": inp["g1_b1"].astype(f32),
        "b_g2h": inp["g2_b1"].astype(f32),
        "b_gw2o": np.concatenate([inp["g1_b2"].astype(f32) * 0.5,
                                  inp["g2_b2"].astype(f32) * 0.5]),
        "b_f": np.concatenate([inp["fl_b"].astype(f32),
                               inp["fa_b"].astype(f32),
                               inp["fv_b"].astype(f32)]),
        "b_o1": inp["o_b1"].astype(f32),
        "b_o2": inp["o_b2"].astype(f32),
    }
    bflat = np.zeros(B_PAD, f32)
    for name, (off, n) in B_LAY.items():
        bflat[off:off + n] = bd[name]
    whb = np.concatenate(
        [hpack, bflat.reshape(128, B_ROW)], axis=1).astype(BF)
    return wpack.astype(F8), whb


def _prep_x(x):
    """x [T, 512, 556] -> [8, XB] fp8 (row c = core c's flat compact pack).

    Per core: main block (p, k', t, r) with k' indexing k in {0,1,3,4};
    k2 block: the 44 valid rows of k-tile 2 as (p2, t, r).
    k-tile rows in the original 556 feature space:
      k0: 0:128, k1: 128:256, k2: 256:300 (x_l tail), k3: 300:428 (x_a),
      k4: 428:556 (x_v).
    """
    out = np.empty((NC, XB), F8)
    for c in range(NC):
        xc = x[:, c * R:(c + 1) * R, :].astype(np.float32)       # [T, 64, 556]
        xt = np.ascontiguousarray(xc.transpose(2, 0, 1))         # [556, T, 64]
        x8 = xt.reshape(556, T * R).astype(F8)
        out[c, 0:XMAIN] = np.concatenate(
            [x8[0:128], x8[128:256], x8[300:428], x8[428:556]],
            axis=0).reshape(-1)
        out[c, XMAIN:] = x8[256:300].reshape(-1)
    return out


def build_nc():
    import concourse.bass as bass
    import concourse.bacc as bacc
    import concourse.tile as tile
    from concourse import mybir, masks
    global F32, BF16
    F32 = mybir.dt.float32
    BF16 = mybir.dt.bfloat16
    FP8 = mybir.dt.float8e4
    AF = mybir.ActivationFunctionType
    ALU = mybir.AluOpType

    nc = bacc.Bacc("TRN2", target_bir_lowering=False, debug=False, num_devices=NC)

    xt_d = nc.dram_tensor("xt8", [1, XB], FP8, kind="ExternalInput").ap()
    w_d = nc.dram_tensor("wsh", [RSH, W_COLS], FP8, kind="ExternalInput").ap()
    wb_d = nc.dram_tensor("whb", [RSH, WHB_COLS], BF16, kind="ExternalInput").ap()
    out_d = nc.dram_tensor("o_out", [R, 1], F32, kind="ExternalOutput").ap()
    dbg_d = None
    dbgb_d = None
    dbgh_d = None
    if DEBUG_HEAD:
        dbg_d = nc.dram_tensor("dbg", [128, 1024], F32, kind="ExternalOutput").ap()
        dbgh_d = nc.dram_tensor("dbgh", [128, H_COLS], BF16, kind="ExternalOutput").ap()
        dbgb_d = nc.dram_tensor("dbgb", [1, B_PAD], BF16, kind="ExternalOutput").ap()

    with TileBuild(nc, tile, mybir, masks, AF, ALU) as b:
        b.run(xt_d, w_d, wb_d, out_d, FP8, dbg_d, dbgb_d, dbgh_d)
    nc.compile()
    return nc


class TileBuild:
    def __init__(self, nc, tile, mybir, masks, AF, ALU):
        self.nc, self.tile, self.mybir = nc, tile, mybir
        self.masks, self.AF, self.ALU = masks, AF, ALU

    def __enter__(self):
        self.ctx = ExitStack()
        self.tc = self.ctx.enter_context(self.tile.TileContext(self.nc))
        return self

    def __exit__(self, *a):
        self.ctx.close()

    def run(self, xt_d, w_d, wb_d, out_d, FP8, dbg_d=None, dbgb_d=None, dbgh_d=None):
        nc, tc, ctx = self.nc, self.tc, self.ctx
        AF, ALU = self.AF, self.ALU
        f32, bf16 = F32, BF16

        const = ctx.enter_context(tc.tile_pool(name="const", bufs=1))
        wpool = ctx.enter_context(tc.tile_pool(name="wpool", bufs=1))
        state = ctx.enter_context(tc.tile_pool(name="state", bufs=1))
        xin = ctx.enter_context(tc.tile_pool(name="xin", bufs=1))
        scr = ctx.enter_context(tc.tile_pool(name="scr", bufs=2))
        stat = ctx.enter_context(tc.tile_pool(name="stat", bufs=3))
        pmm = ctx.enter_context(tc.tile_pool(name="pmm", bufs=3, space="PSUM"))
        ptp = ctx.enter_context(tc.tile_pool(name="ptp", bufs=2, space="PSUM"))
        dram = ctx.enter_context(tc.tile_pool(name="dram", bufs=1, space="DRAM"))

        idf = const.tile([128, 128], f32, tag="idf", name="idf")
        self.masks.make_identity(nc, idf[:])
        idb = const.tile([128, 128], bf16, tag="idb", name="idb")
        self.masks.make_identity(nc, idb[:])
        onesf = const.tile([1, 64], f32, tag="onesf", name="onesf")
        nc.vector.memset(onesf[:], 1.0)

        # ---- weights: fp8 shard -> bounce -> AllGather -> SBUF (fp8 rhs);
        # head weights + biases ride a separate bf16 AllGather.
        win_b = dram.tile([RSH, W_COLS], FP8, tag="win_b", name="win_b")
        wg_b = dram.tile([128, W_COLS], FP8, tag="wg_b", name="wg_b")
        nc.gpsimd.dma_start(win_b[:], w_d[:])
        nc.gpsimd.collective_compute(
            "AllGather",
            self.mybir.AluOpType.bypass,
            replica_groups=[list(range(NC))],
            ins=[win_b[:].opt()],
            outs=[wg_b[:].opt()],
        )
        wsb = wpool.tile([128, W_COLS], FP8, tag="wsb", name="wsb")
        nc.sync.dma_start(wsb[:], wg_b[:])
        wbin_b = dram.tile([RSH, WHB_COLS], bf16, tag="wbin_b", name="wbin_b")
        wbg_b = dram.tile([128, WHB_COLS], bf16, tag="wbg_b", name="wbg_b")
        nc.gpsimd.dma_start(wbin_b[:], wb_d[:])
        nc.gpsimd.collective_compute(
            "AllGather",
            self.mybir.AluOpType.bypass,
            replica_groups=[list(range(NC))],
            ins=[wbin_b[:].opt()],
            outs=[wbg_b[:].opt()],
        )
        hsb = wpool.tile([128, H_COLS], bf16, tag="hsb", name="hsb")
        nc.sync.dma_start(hsb[:], wbg_b[:, 0:H_COLS])
        bsb = wpool.tile([1, B_PAD], bf16, tag="bsb", name="bsb")
        nc.sync.dma_start(bsb[:], wbg_b[:, H_COLS:WHB_COLS])

        def wtile(name, k, cols):
            off, kt, n = W_LAY[name]
            return wsb[:, off + k * n + cols.start: off + k * n + cols.stop]

        def htile(name, k, cols):
            off, kt, n = H_LAY[name]
            return hsb[:, off + k * n + cols.start: off + k * n + cols.stop]

        def btile(name, cols=None):
            off, n = B_LAY[name]
            if cols is None:
                cols = slice(0, n)
            return bsb[0:1, off + cols.start: off + cols.stop]

        ones = btile("ones")

        # ---- x: compact fp8 in, cast to bf16 by gpsimd DMAs. SBUF layout is
        # k-major: col = k*T*R + t*R + r.
        from concourse.ap import AP as _AP
        xsb = xin.tile([128, T * 5 * R], bf16, tag="xsb", name="xsb")
        TR = T * R
        nc.vector.memset(xsb[:, 2 * TR:3 * TR], 0.0)   # k2 pad rows 44:128
        for j, k in enumerate((0, 1, 3, 4)):
            src = _AP(xt_d.tensor, j * 128 * TR,
                      self.mybir.VecI64Pair([[TR, 128], [1, TR]]))
            nc.gpsimd.dma_start(xsb[:, k * TR:(k + 1) * TR], src)
        srck2 = _AP(xt_d.tensor, XMAIN,
                    self.mybir.VecI64Pair([[TR, 44], [1, TR]]))
        nc.gpsimd.dma_start(xsb[0:44, 2 * TR:3 * TR], srck2)

        # persistent state
        Cd = [state.tile([R, 512], f32, tag=f"cd{i}", name=f"cd{i}") for i in range(2)]
        Mem = [state.tile([R, MEM], f32, tag=f"mem{i}", name=f"mem{i}") for i in range(2)]
        H = state.tile([R, 512], f32, tag="H", name="H")
        cT = [state.tile([128, 256], bf16, tag=f"ct{i}", name=f"ct{i}") for i in range(2)]
        hT = state.tile([128, 256], bf16, tag="hT", name="hT")
        memT = state.tile([128, 128], bf16, tag="memT", name="memT")
        for t_ in Cd + Mem + [H]:
            nc.vector.memset(t_[:], 0.0)
        for t_ in cT + [hT, memT]:
            nc.vector.memset(t_[:], 0.0)

        def preload(ps_slice, bias_ap):
            nc.tensor.matmul(ps_slice, ones, bias_ap, start=True, stop=False,
                             skip_group_check=True)

        def mm(ps_slice, lhsT, rhs, stop=False):
            nc.tensor.matmul(ps_slice, lhsT, rhs, start=False, stop=stop,
                             skip_group_check=True)

        for t in range(T):
            old, new = t % 2, (t + 1) % 2
            Cd_o, Cd_n = Cd[old], Cd[new]
            Mem_o, Mem_n = Mem[old], Mem[new]
            cT_o, cT_n = cT[old], cT[new]

            def xT(k):
                o = k * T * R + t * R
                return xsb[:, o:o + R]

            def hTl(k):
                return hT[:, k * 64:(k + 1) * 64]

            # ---- gates psums: gl [64,1024] (l), gav [64,1024] (a|v)
            gl_ps = pmm.tile([R, 1024], f32, tag="pmm", name="gl_ps")
            gav_ps = pmm.tile([R, 1024], f32, tag="pmm", name="gav_ps")
            for c in range(2):
                preload(gl_ps[:, c * 512:(c + 1) * 512],
                        btile("b_g", slice(c * 512, (c + 1) * 512)))
            preload(gav_ps[:, 0:512], btile("b_g", slice(1024, 1536)))
            preload(gav_ps[:, 512:1024], btile("b_g", slice(1536, 2048)))
            for c in range(2):
                sl = gl_ps[:, c * 512:(c + 1) * 512]
                wcols = slice(c * 512, (c + 1) * 512)
                for k in range(2):
                    mm(sl, hTl(k), wtile("wg_l", k, wcols))
                for k in range(3):
                    mm(sl, xT(k), wtile("wx_l", k, wcols), stop=(k == 2))
            mm(gav_ps[:, 0:512], hTl(2), wtile("wg_a", 0, slice(0, 512)))
            mm(gav_ps[:, 0:512], xT(3), wtile("wx_a", 0, slice(0, 512)), stop=True)
            mm(gav_ps[:, 512:1024], hTl(3), wtile("wg_v", 0, slice(0, 512)))
            mm(gav_ps[:, 512:1024], xT(4), wtile("wx_v", 0, slice(0, 512)), stop=True)

            # ---- G = tanh(gates)  (i,f,o prescaled by 0.5 in weights)
            G = scr.tile([R, GATES], f32, tag="G", name="G")
            nc.scalar.activation(G[:, 0:1024], gl_ps[:], AF.Tanh)
            nc.scalar.activation(G[:, 1024:2048], gav_ps[:], AF.Tanh)

            # ---- cell update: D_new = 0.5*(1+tf)*D_old + (1+ti)*tg
            # gate col ranges: l: i 0:256 f 256:512 g 512:768 o 768:1024
            #                  a: i 1024:1152 f .. g .. o 1408:1536 ; v: +512
            q = scr.tile([R, 512], f32, tag="q", name="q")
            p = scr.tile([R, 512], f32, tag="p", name="p")
            GR = {"l": (0, HL), "a": (1024, HA), "v": (1536, HV)}
            off_c = {"l": 0, "a": 256, "v": 384}
            for m_ in "lav":
                g0, h = GR[m_]
                c0 = off_c[m_]
                nc.vector.scalar_tensor_tensor(
                    q[:, c0:c0 + h], G[:, g0:g0 + h], 1.0,
                    G[:, g0 + 2 * h:g0 + 3 * h], ALU.add, ALU.mult)
                nc.vector.scalar_tensor_tensor(
                    p[:, c0:c0 + h], G[:, g0 + h:g0 + 2 * h], 1.0,
                    Cd_o[:, c0:c0 + h], ALU.add, ALU.mult)
            nc.vector.scalar_tensor_tensor(
                Cd_n[:], p[:], 0.5, q[:], ALU.mult, ALU.add)

            # ---- h = (1+to)*tanh(Dnew/2)  (doubled h)
            tc2 = scr.tile([R, 512], f32, tag="tc2", name="tc2")
            nc.scalar.activation(tc2[:], Cd_n[:], AF.Tanh, scale=0.5)
            for m_ in "lav":
                g0, h = GR[m_]
                c0 = off_c[m_]
                nc.vector.scalar_tensor_tensor(
                    H[:, c0:c0 + h], G[:, g0 + 3 * h:g0 + 4 * h], 1.0,
                    tc2[:, c0:c0 + h], ALU.add, ALU.mult)

            # ---- transposes: cT_new + hT (8 chunks) -> one f32 psum + 1 drain
            tp1 = ptp.tile([128, 512], f32, tag="tp", name="tp")
            for i in range(4):
                nc.tensor.transpose(tp1[:, i * 64:(i + 1) * 64],
                                    Cd_n[:, i * 128:(i + 1) * 128], idf[0:64, 0:64])
            for i in range(4):
                nc.tensor.transpose(tp1[:, 256 + i * 64:256 + (i + 1) * 64],
                                    H[:, i * 128:(i + 1) * 128], idf[0:64, 0:64])
            nc.vector.tensor_copy(cT_n[:], tp1[:, 0:256])
            nc.vector.tensor_copy(hT[:], tp1[:, 256:512])

            # ---- att1 hidden: relu(a1w1 @ cStar)
            pa1 = pmm.tile([R, 1024], f32, tag="pmm", name="pmm")
            preload(pa1[:, 0:512], btile("b_a1h"))
            for k in range(8):
                st = cT_o[:, (k % 4) * 64:(k % 4 + 1) * 64] if k < 4 \
                    else cT_n[:, (k - 4) * 64:(k - 3) * 64]
                mm(pa1[:, 0:512], st, wtile("a1w1", k, slice(0, 512)), stop=(k == 7))
            relu1 = scr.tile([R, 512], bf16, tag="relu1", name="relu1")
            nc.vector.tensor_scalar_max(relu1[:], pa1[:, 0:512], 0.0)
            tp2 = ptp.tile([128, 256], bf16, tag="tp", name="tp")
            for i in range(4):
                nc.tensor.transpose(tp2[:, i * 64:(i + 1) * 64],
                                    relu1[:, i * 128:(i + 1) * 128], idb[0:64, 0:64])
            r1T = stat.tile([128, 256], bf16, tag="r1T", name="r1T")
            nc.vector.tensor_copy(r1T[:], tp2[:])

            # ---- logits + softmax (no max-sub; exp then normalize)
            pe2 = pmm.tile([R, 1024], f32, tag="pmm", name="pmm")
            for c in range(2):
                sl = pe2[:, c * 512:(c + 1) * 512]
                preload(sl, btile("b_a1o", slice(c * 512, (c + 1) * 512)))
                for k in range(4):
                    mm(sl, r1T[:, k * 64:(k + 1) * 64],
                       wtile("a1w2", k, slice(c * 512, (c + 1) * 512)), stop=(k == 3))
            E = scr.tile([R, 1024], f32, tag="E", name="E")
            es = scr.tile([R, 1], f32, tag="es", name="es")
            nc.scalar.activation(E[:], pe2[:], AF.Exp, accum_out=es[:])
            recip = scr.tile([R, 1], f32, tag="recip", name="recip")
            nc.vector.reciprocal(recip[:], es[:])

            # ---- attended (doubled) = E * recip * Dstar
            att = scr.tile([R, 1024], bf16, tag="att", name="att")
            nc.vector.scalar_tensor_tensor(att[:, 0:512], E[:, 0:512], recip[:, 0:1],
                                           Cd_o[:], ALU.mult, ALU.mult)
            nc.vector.scalar_tensor_tensor(att[:, 512:1024], E[:, 512:1024],
                                           recip[:, 0:1], Cd_n[:], ALU.mult, ALU.mult)
            tp3 = ptp.tile([128, 512], bf16, tag="tp", name="tp")
            for i in range(8):
                nc.tensor.transpose(tp3[:, i * 64:(i + 1) * 64],
                                    att[:, i * 128:(i + 1) * 128], idb[0:64, 0:64])
            attT = stat.tile([128, 512], bf16, tag="attT", name="attT")
            nc.vector.tensor_copy(attT[:], tp3[:])

            def bothT(k):
                return attT[:, k * 64:(k + 1) * 64] if k < 8 \
                    else memT[:, (k - 8) * 64:(k - 7) * 64]

            # ---- att2 hidden + cHat
            pa2 = pmm.tile([R, 1024], f32, tag="pmm", name="pmm")
            preload(pa2[:, 0:512], btile("b_a2h"))
            for k in range(8):
                mm(pa2[:, 0:512], attT[:, k * 64:(k + 1) * 64],
                   wtile("a2w1", k, slice(0, 512)), stop=(k == 7))
            relu2 = scr.tile([R, 512], bf16, tag="relu2", name="relu2")
            nc.vector.tensor_scalar_max(relu2[:], pa2[:, 0:512], 0.0)
            tp4 = ptp.tile([128, 256], bf16, tag="tp", name="tp")
            for i in range(4):
                nc.tensor.transpose(tp4[:, i * 64:(i + 1) * 64],
                                    relu2[:, i * 128:(i + 1) * 128], idb[0:64, 0:64])
            r2T = stat.tile([128, 256], bf16, tag="r2T", name="r2T")
            nc.vector.tensor_copy(r2T[:], tp4[:])

            pc = pmm.tile([R, 1024], f32, tag="pmm", name="pmm")
            preload(pc[:, 0:256], btile("b_a2o"))
            for k in range(4):
                mm(pc[:, 0:256], r2T[:, k * 64:(k + 1) * 64],
                   wtile("a2w2", k, slice(0, 256)), stop=(k == 3))
            cHat = scr.tile([R, MEM], f32, tag="cHat", name="cHat")
            nc.scalar.activation(cHat[:], pc[:, 0:256], AF.Tanh)

            # ---- g1/g2 hidden
            pgh = pmm.tile([R, 1024], f32, tag="pmm", name="pmm")
            for gi, gn in ((0, "g1w1"), (1, "g2w1")):
                sl = pgh[:, gi * 512:(gi + 1) * 512]
                preload(sl, btile("b_g1h" if gi == 0 else "b_g2h"))
                for k in range(10):
                    mm(sl, bothT(k), wtile(gn, k, slice(0, 512)), stop=(k == 9))
            rg = scr.tile([R, 1024], bf16, tag="rg", name="rg")
            nc.vector.tensor_scalar_max(rg[:], pgh[:], 0.0)
            tp5 = ptp.tile([128, 512], bf16, tag="tp", name="tp")
            for i in range(8):
                nc.tensor.transpose(tp5[:, i * 64:(i + 1) * 64],
                                    rg[:, i * 128:(i + 1) * 128], idb[0:64, 0:64])
            rgT = stat.tile([128, 512], bf16, tag="rgT", name="rgT")
            nc.vector.tensor_copy(rgT[:], tp5[:])

            # ---- gammas (tanh-trick, 0.5 baked into g?w2 + bias)
            pga = pmm.tile([R, 1024], f32, tag="pmm", name="pmm")
            preload(pga[:, 0:512], btile("b_gw2o"))
            for k in range(4):
                mm(pga[:, 0:256], rgT[:, k * 64:(k + 1) * 64],
                   wtile("g1w2", k, slice(0, 256)), stop=(k == 3))
            for k in range(4):
                mm(pga[:, 256:512], rgT[:, 256 + k * 64:256 + (k + 1) * 64],
                   wtile("g2w2", k, slice(0, 256)), stop=(k == 3))
            Tg = scr.tile([R, 512], f32, tag="Tg", name="Tg")
            nc.scalar.activation(Tg[:], pga[:, 0:512], AF.Tanh)
            Gam = scr.tile([R, 512], f32, tag="Gam", name="Gam")
            nc.vector.tensor_scalar(Gam[:], Tg[:], 0.5, 0.5, ALU.mult, ALU.add)

            # ---- mem update
            ma = scr.tile([R, MEM], f32, tag="ma", name="ma")
            nc.vector.tensor_tensor(ma[:], Gam[:, 0:256], Mem_o[:], ALU.mult)
            mb = scr.tile([R, MEM], f32, tag="mb", name="mb")
            nc.vector.tensor_tensor(mb[:], Gam[:, 256:512], cHat[:], ALU.mult)
            nc.vector.tensor_tensor(Mem_n[:], ma[:], mb[:], ALU.add)

            tp6 = ptp.tile([128, 128], f32, tag="tp", name="tp")
            for i in range(2):
                nc.tensor.transpose(tp6[:, i * 64:(i + 1) * 64],
                                    Mem_n[:, i * 128:(i + 1) * 128], idf[0:64, 0:64])
            nc.vector.tensor_copy(memT[:], tp6[:])

        # ================= head (on device) =================
        # logits z_m = (H_m/2) @ fw_m.T + fb_m  (0.5 folded into wf)
        AX = self.mybir.AxisListType.X
        zf = pmm.tile([R, 1024], f32, tag="pmm", name="zf")
        preload(zf[:, 0:384], btile("b_f"))
        mm(zf[:, 0:128], hT[:, 0:64], htile("wf_l", 0, slice(0, 128)))
        mm(zf[:, 0:128], hT[:, 64:128], htile("wf_l", 1, slice(0, 128)),
           stop=True)
        mm(zf[:, 128:256], hT[:, 128:192], htile("wf_a", 0, slice(0, 128)),
           stop=True)
        mm(zf[:, 256:384], hT[:, 192:256], htile("wf_v", 0, slice(0, 128)),
           stop=True)
        zsb = scr.tile([R, 384], f32, tag="zsb", name="zsb")
        nc.vector.tensor_copy(zsb[:], zf[:, 0:384])
        # per-core per-modality max -> [64, 3] -> [3, 1]
        zmax = scr.tile([R, 3], f32, tag="zmax", name="zmax")
        for m in range(3):
            nc.vector.tensor_reduce(zmax[:, m:m + 1],
                                    zsb[:, m * 128:(m + 1) * 128], AX, ALU.max)
        tpm = ptp.tile([128, 64], f32, tag="tp", name="tpm")
        nc.tensor.transpose(tpm[0:3, 0:64], zmax[0:64, 0:3], idf[0:64, 0:64])
        mxc = scr.tile([128, 1], f32, tag="mxc", name="mxc")
        nc.vector.tensor_reduce(mxc[0:3, 0:1], tpm[0:3, 0:64], AX, ALU.max)
        # global max via AllReduce(max), then broadcast to [64, 3]
        mx_in = dram.tile([3, 1], f32, tag="mx_in", name="mx_in")
        mx_out = dram.tile([3, 1], f32, tag="mx_out", name="mx_out")
        nc.gpsimd.dma_start(mx_in[:], mxc[0:3, 0:1])
        nc.gpsimd.collective_compute(
            "AllReduce", ALU.max, replica_groups=[list(range(NC))],
            ins=[mx_in[:].opt()], outs=[mx_out[:].opt()])
        mrow = scr.tile([1, 3], f32, tag="mrow", name="mrow")
        nc.sync.dma_start(mrow[:], mx_out[:])
        pb = ptp.tile([128, 64], f32, tag="tp", name="pb")
        nc.tensor.matmul(pb[0:64, 0:3], onesf[:], mrow[:], start=True,
                         stop=True, skip_group_check=True)
        nmax = scr.tile([R, 3], f32, tag="nmax", name="nmax")
        nc.vector.tensor_scalar(nmax[:], pb[0:64, 0:3], -1.0, None, ALU.mult)
        # p = exp(z - max); u = z - max; lav = sum_m p_m * u_m
        Ex = scr.tile([R, 384], f32, tag="Ex", name="Ex")
        uu = scr.tile([R, 384], f32, tag="uu", name="uu")
        for m in range(3):
            sl = slice(m * 128, (m + 1) * 128)
            nc.scalar.activation(Ex[:, sl], zsb[:, sl], AF.Exp,
                                 bias=nmax[:, m:m + 1])
            nc.vector.tensor_scalar(uu[:, sl], zsb[:, sl], nmax[:, m:m + 1],
                                    None, ALU.add)
        pu = scr.tile([R, 384], f32, tag="pu", name="pu")
        nc.vector.tensor_tensor(pu[:], Ex[:], uu[:], ALU.mult)
        lv1 = scr.tile([R, 128], f32, tag="lv1", name="lv1")
        nc.vector.tensor_tensor(lv1[:], pu[:, 0:128], pu[:, 128:256], ALU.add)
        lav = scr.tile([R, 128], bf16, tag="lav", name="lav")
        nc.vector.tensor_tensor(lav[:], lv1[:], pu[:, 256:384], ALU.add)
        tpl = ptp.tile([128, 64], bf16, tag="tp", name="tpl")
        nc.tensor.transpose(tpl[:, 0:64], lav[:, 0:128], idb[0:64, 0:64])
        lavT = stat.tile([128, 64], bf16, tag="lavT", name="lavT")
        nc.vector.tensor_copy(lavT[:], tpl[:])
        # o1 = relu([lav | mem] @ o_w1.T + o_b1)   [64, 256]
        po = pmm.tile([R, 1024], f32, tag="pmm", name="po")
        preload(po[:, 0:256], btile("b_o1"))
        mm(po[:, 0:256], lavT[:, 0:64], htile("wo1", 0, slice(0, 256)))
        mm(po[:, 0:256], memT[:, 0:64], htile("wo1", 1, slice(0, 256)))
        mm(po[:, 0:256], memT[:, 64:128], htile("wo1", 2, slice(0, 256)),
           stop=True)
        o1r = scr.tile([R, 256], bf16, tag="o1r", name="o1r")
        nc.vector.tensor_scalar_max(o1r[:], po[:, 0:256], 0.0)
        tpo = ptp.tile([128, 128], bf16, tag="tp", name="tpo")
        for i in range(2):
            nc.tensor.transpose(tpo[:, i * 64:(i + 1) * 64],
                                o1r[:, i * 128:(i + 1) * 128], idb[0:64, 0:64])
        o1T = stat.tile([128, 128], bf16, tag="o1T", name="o1T")
        nc.vector.tensor_copy(o1T[:], tpo[:])
        poz = pmm.tile([R, 1024], f32, tag="pmm", name="poz")
        preload(poz[:, 0:1], btile("b_o2"))
        for i in range(2):
            mm(poz[:, 0:1], o1T[:, i * 64:(i + 1) * 64],
               htile("wo2", i, slice(0, 1)), stop=(i == 1))
        outsb = scr.tile([R, 1], f32, tag="outsb", name="outsb")
        nc.vector.tensor_copy(outsb[:], poz[:, 0:1])
        nc.sync.dma_start(out_d[:], outsb[:])
        if dbg_d is not None:
            # rows 0:64: [zsb 0:384 | nmax 384:387 | lav(f32) 512:640]
            # rows 0:128: [hT 640:896 | memT 896:1024] (bf16 upcast)
            nc.sync.dma_start(dbg_d[0:64, 0:384], zsb[:])
            nc.sync.dma_start(dbg_d[0:64, 384:387], nmax[:])
            lavf = scr.tile([R, 128], f32, tag="lavf", name="lavf")
            nc.vector.tensor_copy(lavf[:], lav[:])
            nc.sync.dma_start(dbg_d[0:64, 512:640], lavf[:])
            hTf = scr.tile([128, 384], f32, tag="hTf", name="hTf")
            nc.vector.tensor_copy(hTf[:, 0:256], hT[:])
            nc.vector.tensor_copy(hTf[:, 256:384], memT[:])
            nc.sync.dma_start(dbg_d[:, 640:1024], hTf[:])
            o1f = scr.tile([R, 256], f32, tag="o1f", name="o1f")
            nc.vector.tensor_copy(o1f[:], o1r[:])
            nc.sync.dma_start(dbg_d[64:128, 0:256], o1f[:])
            nc.sync.dma_start(dbgh_d[:], hsb[:])
            nc.sync.dma_start(dbgb_d[:], bsb[:])


_CACHED = {}


def _build_executor(nc):
    """Build (once) a cached jit'd shard_map executable mirroring
    bass2jax.run_bass_via_pjrt, so repeat calls skip retrace/relower."""
    import jax
    from jax.sharding import Mesh, PartitionSpec
    from jax.experimental.shard_map import shard_map
    from concourse import mybir
    from concourse.bass2jax import (_bass_exec_p, install_neuronx_cc_hook,
                                    partition_id_tensor)

    install_neuronx_cc_hook()

    partition_name = nc.partition_id_tensor.name if nc.partition_id_tensor else None
    in_names, out_names, out_avals = [], [], []
    for alloc in nc.m.functions[0].allocations:
        if not isinstance(alloc, mybir.MemoryLocationSet):
            continue
        name = alloc.memorylocations[0].name
        if alloc.kind == "ExternalInput":
            if name != partition_name:
                in_names.append(name)
        elif alloc.kind == "ExternalOutput":
            out_names.append(name)
            out_avals.append(jax.core.ShapedArray(
                tuple(alloc.tensor_shape), mybir.dt.np(alloc.dtype)))
    n_params = len(in_names)
    n_outs = len(out_names)
    all_in_names = in_names + out_names + ([partition_name] if partition_name else [])

    def _body(*args):
        operands = list(args)
        if partition_name is not None:
            operands.append(partition_id_tensor())
        outs = _bass_exec_p.bind(
            *operands,
            out_avals=tuple(out_avals),
            in_names=tuple(all_in_names),
            out_names=tuple(out_names),
            lowering_input_output_aliases=(),
            sim_require_finite=True,
            sim_require_nnan=True,
            nc=nc,
        )
        return tuple(outs)

    devices = jax.devices()[:NC]
    mesh = Mesh(np.asarray(devices), ("core",))
    in_specs = (PartitionSpec("core"),) * (n_params + n_outs)
    out_specs = (PartitionSpec("core"),) * n_outs
    donate = tuple(range(n_params, n_params + n_outs))
    sharded = jax.jit(
        shard_map(_body, mesh=mesh, in_specs=in_specs, out_specs=out_specs,
                  check_rep=False),
        donate_argnums=donate, keep_unused=True)

    def execute(arrs):
        """arrs: dict name -> global [NC*d0, ...] array (sharded on axis 0)."""
        concat_in = [arrs[n] for n in in_names]
        zeros = [np.zeros((NC * a.shape[0], *a.shape[1:]), a.dtype)
                 for a in out_avals]
        outs = sharded(*concat_in, *zeros)
        host = [np.asarray(o) for o in outs]
        return [
            {name: host[i].reshape(NC, *out_avals[i].shape)[c]
             for i, name in enumerate(out_names)}
            for c in range(NC)
        ]

    return execute


def _get_executor():
    if "exec" not in _CACHED:
        _CACHED["nc"] = build_nc()
        _CACHED["exec"] = _build_executor(_CACHED["nc"])
    return _CACHED["exec"]


def _postprocess(res, inputs):
    """The head runs on device; just stitch the per-core outputs."""
    return np.concatenate(
        [res[c]["o_out"] for c in range(NC)], axis=0).flatten().astype(np.float32)


def kernel(**inputs):
    x = np.asarray(inputs["x"])
    w8, whb = _prep_params(inputs)
    xall = _prep_x(x)
    execute = _get_executor()
    res = execute({"xt8": xall, "wsh": w8, "whb": whb})
    return _postprocess(res, inputs)


# revision 37
# speedup vs baseline: 10.3745x; 1.0228x over previous
"""MFN (Memory Fusion Network) Trainium2 Bass kernel.

Strategy: data-parallel over batch (512 -> 8 cores x 64 rows). Per core the
whole T=64 recurrence runs on-chip: PE matmuls with bf16 stationary
activations ([K,64] transposes) against fp8-e4m3 streamed weights, fp32
elementwise, PSUM fp32 accumulation. sigmoid is computed via
sigma(x) = 0.5 + 0.5*tanh(x/2) (the 1/2 baked into weights) so the whole
kernel uses only the exp+tanh ACT tables and never pays table switches.
Cell state and h are kept doubled (D = 2c, H = 2h), with the compensating
0.5 factors folded into the prepped weight matrices. The final head
(3 logit matmuls, global-max exp fusion lav = sum_m p_m*(z_m-M_m), and the
2-layer output MLP) also runs on device; the global max is one tiny
AllReduce(max). Head weights stay bf16 (the final dot has heavy
cancellation; fp8 there fails the tolerance).

Wall-clock under axon is dominated by shipping inputs over the tunnel
(~65-120 MB/s) plus a ~120 ms fixed dispatch+fetch floor; device exec is
nearly free. Hence:
  - x ships as fp8 e4m3 in a compact flat pack (no 556->640 zero padding;
    18.2 MB total) and is cast to bf16 by gpsimd DMAs on-chip.
  - LSTM/attention/gate weights ship as fp8 shards (1/8 per core, 0.5 MB)
    riding in the same fp8 array as x, AllGather'd on device. Head weights
    + biases ride a separate small bf16 array with its own AllGather —
    NOT byte-packed into the fp8 array, because the transport canonicalizes
    fp8-NaN byte patterns (0x79-0x7F/0xF9-0xFF -> 0x7C) on remote legs.
  - the output is just [64,1] per core (the full head runs on device).
  - the jit'd shard_map executable is built once and cached (bass_utils
    re-traces on every call, costing ~1.3 s).
"""
import numpy as np
import ml_dtypes
from contextlib import ExitStack

BF = ml_dtypes.bfloat16
DEBUG_HEAD = False
F8 = ml_dtypes.float8_e4m3

# model dims (hardcoded from the problem spec)
T, NFULL, DIN = 64, 512, 556
DL, DA, DV = 300, 128, 128
HL, HA, HV = 256, 128, 128
DLP = 384                     # DL padded to 3*128
DP = DLP + DA + DV            # 640 padded input feature dim
R = 64                        # batch rows per core
NC = 8
ATT_IN = 1024
H1 = H2 = HG = 512
MEM = 256
GATES = 4 * HL + 4 * HA + 4 * HV   # 2048

F32 = None
BF16 = None


def _w_layout():
    """Column offsets of each prepped K-tiled weight inside wpack [128, W]."""
    specs = {
        # name: (K, N)
        "wg_l": (HL, 1024), "wg_a": (HA, 512), "wg_v": (HV, 512),
        "wx_l": (DLP, 1024), "wx_a": (DA, 512), "wx_v": (DV, 512),
        "a1w1": (ATT_IN, H1), "a1w2": (H1, ATT_IN),
        "a2w1": (ATT_IN, H2), "a2w2": (H2, MEM),
        "g1w1": (ATT_IN + MEM, HG), "g2w1": (ATT_IN + MEM, HG),
        "g1w2": (HG, MEM), "g2w2": (HG, MEM),
    }
    off, out = 0, {}
    for name, (k, n) in specs.items():
        kt = (k + 127) // 128
        out[name] = (off, kt, n)
        off += kt * n
    return out, off


def _h_layout():
    """Head weights stay bf16 (final dot has heavy cancellation; fp8 fails)."""
    H_OUT = 256
    specs = {
        "wf_l": (HL, 128), "wf_a": (HA, 128), "wf_v": (HV, 128),
        "wo1": (128 + MEM, H_OUT), "wo2": (H_OUT, 1),
    }
    off, out = 0, {}
    for name, (k, n) in specs.items():
        kt = (k + 127) // 128
        out[name] = (off, kt, n)
        off += kt * n
    return out, off


def _b_layout():
    specs = {
        "ones": 64, "b_g": GATES, "b_a1h": H1, "b_a1o": ATT_IN,
        "b_a2h": H2, "b_a2o": MEM, "b_g1h": HG, "b_g2h": HG, "b_gw2o": 2 * MEM,
        "b_f": 3 * 128, "b_o1": 256, "b_o2": 1,
    }
    off, out = 0, {}
    for name, n in specs.items():
        out[name] = (off, n)
        off += n
    return out, off


W_LAY, W_COLS = _w_layout()
H_LAY, H_COLS = _h_layout()
B_LAY, B_COLS = _b_layout()
B_PAD = ((B_COLS + 127) // 128) * 128     # bias pack padded
B_ROW = B_PAD // 128                      # bias bf16 elements per row
WHB_COLS = H_COLS + B_ROW                 # bf16 head+bias shard row length
RSH = 128 // NC                           # 16 rows per core shard

# compact x: k-major flat layout. main block (p, k', t, r) for k' over
# k in {0,1,3,4}; k2 block (44 valid rows only) appended.
XMAIN = 128 * 4 * T * R                   # 2097152
XK2 = 44 * T * R                          # 180224
XB = XMAIN + XK2


def _prep_params(inp):
    """Host-side weight prep -> (w8 [128, W_COLS] fp8, whb [128, WHB_COLS]
    bf16). Row-sharded 1/8 per core by shard_map; AllGather'd on device.
    Head weights + biases travel as genuine bf16 (a byte-packed fp8 ride
    corrupts them: the transport canonicalizes fp8-NaN byte patterns)."""
    f32 = np.float32

    def gate_scale_cols(w):      # w: [4h, k] torch layout -> scale i,f,o rows by 0.5
        w = w.astype(f32).copy()
        h = w.shape[0] // 4
        w[0:2 * h] *= 0.5        # i, f
        w[3 * h:4 * h] *= 0.5    # o
        return w

    wd = {}
    # LSTM weights. Whh additionally *0.5 on input (h stored doubled).
    for m, h, d, dpad in (("l", HL, DL, DLP), ("a", HA, DA, DA), ("v", HV, DV, DV)):
        whh = gate_scale_cols(inp[f"Whh_{m}"]) * 0.5          # [4h, h]
        wih = gate_scale_cols(inp[f"Wih_{m}"])                # [4h, d]
        if dpad != d:
            wih = np.concatenate([wih, np.zeros((4 * h, dpad - d), f32)], axis=1)
        wd[f"wg_{m}"] = whh.T                                  # [h, 4h]
        wd[f"wx_{m}"] = wih.T                                  # [dpad, 4h]
    wd["a1w1"] = inp["att1_w1"].astype(f32).T * 0.5            # rows: cStar doubled
    wd["a1w2"] = inp["att1_w2"].astype(f32).T
    wd["a2w1"] = inp["att2_w1"].astype(f32).T * 0.5            # attended doubled
    wd["a2w2"] = inp["att2_w2"].astype(f32).T
    for g in ("g1", "g2"):
        w1 = inp[f"{g}_w1"].astype(f32).T.copy()               # [1280, 512]
        w1[0:ATT_IN] *= 0.5                                    # attended part doubled
        wd[f"{g}w1"] = w1
        wd[f"{g}w2"] = inp[f"{g}_w2"].astype(f32).T * 0.5      # gamma tanh-trick

    hd = {
        # logits weights; 0.5 folded (h stored doubled)
        "wf_l": inp["fl_w"].astype(f32).T * 0.5,
        "wf_a": inp["fa_w"].astype(f32).T * 0.5,
        "wf_v": inp["fv_w"].astype(f32).T * 0.5,
        "wo1": inp["o_w1"].astype(f32).T,                      # [384, 512]
        "wo2": inp["o_w2"].astype(f32).T,                      # [512, 1]
    }

    def pack(lay, cols, src):
        out = np.zeros((128, cols), f32)
        for name, (off, kt, n) in lay.items():
            w = src[name]
            k = w.shape[0]
            wkt = np.zeros((kt * 128, n), f32)
            wkt[:k] = w
            out[:, off:off + kt * n] = \
                wkt.reshape(kt, 128, n).transpose(1, 0, 2).reshape(128, kt * n)
        return out

    wpack = pack(W_LAY, W_COLS, wd)
    hpack = pack(H_LAY, H_COLS, hd)

    def gate_scale_b(b):
        b = b.astype(f32).copy()
        h = b.shape[0] // 4
        b[0:2 * h] *= 0.5
        b[3 * h:] *= 0.5
        return b

    bd = {
        "ones": np.ones(64, f32),
        "b_g": np.concatenate([gate_scale_b(inp[f"bih_{m}"] + inp[f"bhh_{m}"])
                               for m in "lav"]),
        "b_a1h": inp["att1_b1"].astype(f32),
        "b_a1o": inp["att1_b2"].astype(f32),
        "b_a2h": inp["att2_b1"].astype(f32),
        "b_a2o": inp["att2_b2"].astype(f32),
        "b_g1h": inp["g1_b1"].astype(f32),
        "b_g2h": inp["g2_b1"].astype(f32),
        "b_gw2o": np.concatenate([inp["g1_b2"].astype(f32) * 0.5,
                                  inp["g2_b2"].astype(f32) * 0.5]),
        "b_f": np.concatenate([inp["fl_b"].astype(f32),
                               inp["fa_b"].astype(f32),
                               inp["fv_b"].astype(f32)]),
        "b_o1": inp["o_b1"].astype(f32),
        "b_o2": inp["o_b2"].astype(f32),
    }
    bflat = np.zeros(B_PAD, f32)
    for name, (off, n) in B_LAY.items():
        bflat[off:off + n] = bd[name]
    whb = np.concatenate(
        [hpack, bflat.reshape(128, B_ROW)], axis=1).astype(BF)
    return wpack.astype(F8), whb


def _prep_x(x):
    """x [T, 512, 556] -> [8, XB] fp8 (row c = core c's flat compact pack).

    Per core: main block (p, k', t, r) with k' indexing k in {0,1,3,4};
    k2 block: the 44 valid rows of k-tile 2 as (p2, t, r).
    k-tile rows in the original 556 feature space:
      k0: 0:128, k1: 128:256, k2: 256:300 (x_l tail), k3: 300:428 (x_a),
      k4: 428:556 (x_v).
    """
    out = np.empty((NC, XB), F8)
    for c in range(NC):
        xc = x[:, c * R:(c + 1) * R, :].astype(np.float32)       # [T, 64, 556]
        xt = np.ascontiguousarray(xc.transpose(2, 0, 1))         # [556, T, 64]
        x8 = xt.reshape(556, T * R).astype(F8)
        out[c, 0:XMAIN] = np.concatenate(
            [x8[0:128], x8[128:256], x8[300:428], x8[428:556]],
            axis=0).reshape(-1)
        out[c, XMAIN:] = x8[256:300].reshape(-1)
    return out


def build_nc():
    import concourse.bass as bass
    import concourse.bacc as bacc
    import concourse.tile as tile
    from concourse import mybir, masks
    global F32, BF16
    F32 = mybir.dt.float32
    BF16 = mybir.dt.bfloat16
    FP8 = mybir.dt.float8e4
    AF = mybir.ActivationFunctionType
    ALU = mybir.AluOpType

    nc = bacc.Bacc("TRN2", target_bir_lowering=False, debug=False, num_devices=NC)

    xt_d = nc.dram_tensor("xt8", [1, XB], FP8, kind="ExternalInput").ap()
    w_d = nc.dram_tensor("wsh", [RSH, W_COLS], FP8, kind="ExternalInput").ap()
    wb_d = nc.dram_tensor("whb", [RSH, WHB_COLS], BF16, kind="ExternalInput").ap()
    out_d = nc.dram_tensor("o_out", [R, 1], F32, kind="ExternalOutput").ap()
    dbg_d = None
    dbgb_d = None
    dbgh_d = None
    if DEBUG_HEAD:
        dbg_d = nc.dram_tensor("dbg", [128, 1024], F32, kind="ExternalOutput").ap()
        dbgh_d = nc.dram_tensor("dbgh", [128, H_COLS], BF16, kind="ExternalOutput").ap()
        dbgb_d = nc.dram_tensor("dbgb", [1, B_PAD], BF16, kind="ExternalOutput").ap()

    with TileBuild(nc, tile, mybir, masks, AF, ALU) as b:
        b.run(xt_d, w_d, wb_d, out_d, FP8, dbg_d, dbgb_d, dbgh_d)
    nc.compile()
    return nc


class TileBuild:
    def __init__(self, nc, tile, mybir, masks, AF, ALU):
        self.nc, self.tile, self.mybir = nc, tile, mybir
        self.masks, self.AF, self.ALU = masks, AF, ALU

    def __enter__(self):
        self.ctx = ExitStack()
        self.tc = self.ctx.enter_context(self.tile.TileContext(self.nc))
        return self

    def __exit__(self, *a):
        self.ctx.close()

    def run(self, xt_d, w_d, wb_d, out_d, FP8, dbg_d=None, dbgb_d=None, dbgh_d=None):
        nc, tc, ctx = self.nc, self.tc, self.ctx
        AF, ALU = self.AF, self.ALU
        f32, bf16 = F32, BF16

        const = ctx.enter_context(tc.tile_pool(name="const", bufs=1))
        wpool = ctx.enter_context(tc.tile_pool(name="wpool", bufs=1))
        state = ctx.enter_context(tc.tile_pool(name="state", bufs=1))
        xin = ctx.enter_context(tc.tile_pool(name="xin", bufs=1))
        scr = ctx.enter_context(tc.tile_pool(name="scr", bufs=2))
        stat = ctx.enter_context(tc.tile_pool(name="stat", bufs=3))
        pmm = ctx.enter_context(tc.tile_pool(name="pmm", bufs=3, space="PSUM"))
        ptp = ctx.enter_context(tc.tile_pool(name="ptp", bufs=2, space="PSUM"))
        dram = ctx.enter_context(tc.tile_pool(name="dram", bufs=1, space="DRAM"))

        idf = const.tile([128, 128], f32, tag="idf", name="idf")
        self.masks.make_identity(nc, idf[:])
        idb = const.tile([128, 128], bf16, tag="idb", name="idb")
        self.masks.make_identity(nc, idb[:])
        onesf = const.tile([1, 64], f32, tag="onesf", name="onesf")
        nc.vector.memset(onesf[:], 1.0)

        # ---- weights: fp8 shard -> bounce -> AllGather -> SBUF (fp8 rhs);
        # head weights + biases ride a separate bf16 AllGather.
        win_b = dram.tile([RSH, W_COLS], FP8, tag="win_b", name="win_b")
        wg_b = dram.tile([128, W_COLS], FP8, tag="wg_b", name="wg_b")
        nc.gpsimd.dma_start(win_b[:], w_d[:])
        nc.gpsimd.collective_compute(
            "AllGather",
            self.mybir.AluOpType.bypass,
            replica_groups=[list(range(NC))],
            ins=[win_b[:].opt()],
            outs=[wg_b[:].opt()],
        )
        wsb = wpool.tile([128, W_COLS], FP8, tag="wsb", name="wsb")
        nc.sync.dma_start(wsb[:], wg_b[:])
        wbin_b = dram.tile([RSH, WHB_COLS], bf16, tag="wbin_b", name="wbin_b")
        wbg_b = dram.tile([128, WHB_COLS], bf16, tag="wbg_b", name="wbg_b")
        nc.gpsimd.dma_start(wbin_b[:], wb_d[:])
        nc.gpsimd.collective_compute(
            "AllGather",
            self.mybir.AluOpType.bypass,
            replica_groups=[list(range(NC))],
            ins=[wbin_b[:].opt()],
            outs=[wbg_b[:].opt()],
        )
        hsb = wpool.tile([128, H_COLS], bf16, tag="hsb", name="hsb")
        nc.sync.dma_start(hsb[:], wbg_b[:, 0:H_COLS])
        bsb = wpool.tile([1, B_PAD], bf16, tag="bsb", name="bsb")
        nc.sync.dma_start(bsb[:], wbg_b[:, H_COLS:WHB_COLS])

        def wtile(name, k, cols):
            off, kt, n = W_LAY[name]
            return wsb[:, off + k * n + cols.start: off + k * n + cols.stop]

        def htile(name, k, cols):
            off, kt, n = H_LAY[name]
            return hsb[:, off + k * n + cols.start: off + k * n + cols.stop]

        def btile(name, cols=None):
            off, n = B_LAY[name]
            if cols is None:
                cols = slice(0, n)
            return bsb[0:1, off + cols.start: off + cols.stop]

        ones = btile("ones")

        # ---- x: compact fp8 in, cast to bf16 by gpsimd DMAs. SBUF layout is
        # k-major: col = k*T*R + t*R + r.
        from concourse.ap import AP as _AP
        xsb = xin.tile([128, T * 5 * R], bf16, tag="xsb", name="xsb")
        TR = T * R
        nc.vector.memset(xsb[:, 2 * TR:3 * TR], 0.0)   # k2 pad rows 44:128
        for j, k in enumerate((0, 1, 3, 4)):
            src = _AP(xt_d.tensor, j * 128 * TR,
                      self.mybir.VecI64Pair([[TR, 128], [1, TR]]))
            nc.gpsimd.dma_start(xsb[:, k * TR:(k + 1) * TR], src)
        srck2 = _AP(xt_d.tensor, XMAIN,
                    self.mybir.VecI64Pair([[TR, 44], [1, TR]]))
        nc.gpsimd.dma_start(xsb[0:44, 2 * TR:3 * TR], srck2)

        # persistent state
        Cd = [state.tile([R, 512], f32, tag=f"cd{i}", name=f"cd{i}") for i in range(2)]
        Mem = [state.tile([R, MEM], f32, tag=f"mem{i}", name=f"mem{i}") for i in range(2)]
        H = state.tile([R, 512], f32, tag="H", name="H")
        cT = [state.tile([128, 256], bf16, tag=f"ct{i}", name=f"ct{i}") for i in range(2)]
        hT = state.tile([128, 256], bf16, tag="hT", name="hT")
        memT = state.tile([128, 128], bf16, tag="memT", name="memT")
        for t_ in Cd + Mem + [H]:
            nc.vector.memset(t_[:], 0.0)
        for t_ in cT + [hT, memT]:
            nc.vector.memset(t_[:], 0.0)

        def preload(ps_slice, bias_ap):
            nc.tensor.matmul(ps_slice, ones, bias_ap, start=True, stop=False,
                             skip_group_check=True)

        def mm(ps_slice, lhsT, rhs, stop=False):
            nc.tensor.matmul(ps_slice, lhsT, rhs, start=False, stop=stop,
                             skip_group_check=True)

        for t in range(T):
            old, new = t % 2, (t + 1) % 2
            Cd_o, Cd_n = Cd[old], Cd[new]
            Mem_o, Mem_n = Mem[old], Mem[new]
            cT_o, cT_n = cT[old], cT[new]

            def xT(k):
                o = k * T * R + t * R
                return xsb[:, o:o + R]

            def hTl(k):
                return hT[:, k * 64:(k + 1) * 64]

            # ---- gates psums: gl [64,1024] (l), gav [64,1024] (a|v)
            gl_ps = pmm.tile([R, 1024], f32, tag="pmm", name="gl_ps")
            gav_ps = pmm.tile([R, 1024], f32, tag="pmm", name="gav_ps")
            for c in range(2):
                preload(gl_ps[:, c * 512:(c + 1) * 512],
                        btile("b_g", slice(c * 512, (c + 1) * 512)))
            preload(gav_ps[:, 0:512], btile("b_g", slice(1024, 1536)))
            preload(gav_ps[:, 512:1024], btile("b_g", slice(1536, 2048)))
            for c in range(2):
                sl = gl_ps[:, c * 512:(c + 1) * 512]
                wcols = slice(c * 512, (c + 1) * 512)
                for k in range(2):
                    mm(sl, hTl(k), wtile("wg_l", k, wcols))
                for k in range(3):
                    mm(sl, xT(k), wtile("wx_l", k, wcols), stop=(k == 2))
            mm(gav_ps[:, 0:512], hTl(2), wtile("wg_a", 0, slice(0, 512)))
            mm(gav_ps[:, 0:512], xT(3), wtile("wx_a", 0, slice(0, 512)), stop=True)
            mm(gav_ps[:, 512:1024], hTl(3), wtile("wg_v", 0, slice(0, 512)))
            mm(gav_ps[:, 512:1024], xT(4), wtile("wx_v", 0, slice(0, 512)), stop=True)

            # ---- G = tanh(gates)  (i,f,o prescaled by 0.5 in weights)
            G = scr.tile([R, GATES], f32, tag="G", name="G")
            nc.scalar.activation(G[:, 0:1024], gl_ps[:], AF.Tanh)
            nc.scalar.activation(G[:, 1024:2048], gav_ps[:], AF.Tanh)

            # ---- cell update: D_new = 0.5*(1+tf)*D_old + (1+ti)*tg
            # gate col ranges: l: i 0:256 f 256:512 g 512:768 o 768:1024
            #                  a: i 1024:1152 f .. g .. o 1408:1536 ; v: +512
            q = scr.tile([R, 512], f32, tag="q", name="q")
            p = scr.tile([R, 512], f32, tag="p", name="p")
            GR = {"l": (0, HL), "a": (1024, HA), "v": (1536, HV)}
            off_c = {"l": 0, "a": 256, "v": 384}
            for m_ in "lav":
                g0, h = GR[m_]
                c0 = off_c[m_]
                nc.vector.scalar_tensor_tensor(
                    q[:, c0:c0 + h], G[:, g0:g0 + h], 1.0,
                    G[:, g0 + 2 * h:g0 + 3 * h], ALU.add, ALU.mult)
                nc.vector.scalar_tensor_tensor(
                    p[:, c0:c0 + h], G[:, g0 + h:g0 + 2 * h], 1.0,
                    Cd_o[:, c0:c0 + h], ALU.add, ALU.mult)
            nc.vector.scalar_tensor_tensor(
                Cd_n[:], p[:], 0.5, q[:], ALU.mult, ALU.add)

            # ---- h = (1+to)*tanh(Dnew/2)  (doubled h)
            tc2 = scr.tile([R, 512], f32, tag="tc2", name="tc2")
            nc.scalar.activation(tc2[:], Cd_n[:], AF.Tanh, scale=0.5)
            for m_ in "lav":
                g0, h = GR[m_]
                c0 = off_c[m_]
                nc.vector.scalar_tensor_tensor(
                    H[:, c0:c0 + h], G[:, g0 + 3 * h:g0 + 4 * h], 1.0,
                    tc2[:, c0:c0 + h], ALU.add, ALU.mult)

            # ---- transposes: cT_new + hT (8 chunks) -> one f32 psum + 1 drain
            tp1 = ptp.tile([128, 512], f32, tag="tp", name="tp")
            for i in range(4):
                nc.tensor.transpose(tp1[:, i * 64:(i + 1) * 64],
                                    Cd_n[:, i * 128:(i + 1) * 128], idf[0:64, 0:64])
            for i in range(4):
                nc.tensor.transpose(tp1[:, 256 + i * 64:256 + (i + 1) * 64],
                                    H[:, i * 128:(i + 1) * 128], idf[0:64, 0:64])
            nc.vector.tensor_copy(cT_n[:], tp1[:, 0:256])
            nc.vector.tensor_copy(hT[:], tp1[:, 256:512])

            # ---- att1 hidden: relu(a1w1 @ cStar)
            pa1 = pmm.tile([R, 1024], f32, tag="pmm", name="pmm")
            preload(pa1[:, 0:512], btile("b_a1h"))
            for k in range(8):
                st = cT_o[:, (k % 4) * 64:(k % 4 + 1) * 64] if k < 4 \
                    else cT_n[:, (k - 4) * 64:(k - 3) * 64]
                mm(pa1[:, 0:512], st, wtile("a1w1", k, slice(0, 512)), stop=(k == 7))
            relu1 = scr.tile([R, 512], bf16, tag="relu1", name="relu1")
            nc.vector.tensor_scalar_max(relu1[:], pa1[:, 0:512], 0.0)
            tp2 = ptp.tile([128, 256], bf16, tag="tp", name="tp")
            for i in range(4):
                nc.tensor.transpose(tp2[:, i * 64:(i + 1) * 64],
                                    relu1[:, i * 128:(i + 1) * 128], idb[0:64, 0:64])
            r1T = stat.tile([128, 256], bf16, tag="r1T", name="r1T")
            nc.vector.tensor_copy(r1T[:], tp2[:])

            # ---- logits + softmax (no max-sub; exp then normalize)
            pe2 = pmm.tile([R, 1024], f32, tag="pmm", name="pmm")
            for c in range(2):
                sl = pe2[:, c * 512:(c + 1) * 512]
                preload(sl, btile("b_a1o", slice(c * 512, (c + 1) * 512)))
                for k in range(4):
                    mm(sl, r1T[:, k * 64:(k + 1) * 64],
                       wtile("a1w2", k, slice(c * 512, (c + 1) * 512)), stop=(k == 3))
            E = scr.tile([R, 1024], f32, tag="E", name="E")
            es = scr.tile([R, 1], f32, tag="es", name="es")
            nc.scalar.activation(E[:], pe2[:], AF.Exp, accum_out=es[:])
            recip = scr.tile([R, 1], f32, tag="recip", name="recip")
            nc.vector.reciprocal(recip[:], es[:])

            # ---- attended (doubled) = E * recip * Dstar
            att = scr.tile([R, 1024], bf16, tag="att", name="att")
            nc.vector.scalar_tensor_tensor(att[:, 0:512], E[:, 0:512], recip[:, 0:1],
                                           Cd_o[:], ALU.mult, ALU.mult)
            nc.vector.scalar_tensor_tensor(att[:, 512:1024], E[:, 512:1024],
                                           recip[:, 0:1], Cd_n[:], ALU.mult, ALU.mult)
            tp3 = ptp.tile([128, 512], bf16, tag="tp", name="tp")
            for i in range(8):
                nc.tensor.transpose(tp3[:, i * 64:(i + 1) * 64],
                                    att[:, i * 128:(i + 1) * 128], idb[0:64, 0:64])
            attT = stat.tile([128, 512], bf16, tag="attT", name="attT")
            nc.vector.tensor_copy(attT[:], tp3[:])

            def bothT(k):
                return attT[:, k * 64:(k + 1) * 64] if k < 8 \
                    else memT[:, (k - 8) * 64:(k - 7) * 64]

            # ---- att2 hidden + cHat
            pa2 = pmm.tile([R, 1024], f32, tag="pmm", name="pmm")
            preload(pa2[:, 0:512], btile("b_a2h"))
            for k in range(8):
                mm(pa2[:, 0:512], attT[:, k * 64:(k + 1) * 64],
                   wtile("a2w1", k, slice(0, 512)), stop=(k == 7))
            relu2 = scr.tile([R, 512], bf16, tag="relu2", name="relu2")
            nc.vector.tensor_scalar_max(relu2[:], pa2[:, 0:512], 0.0)
            tp4 = ptp.tile([128, 256], bf16, tag="tp", name="tp")
            for i in range(4):
                nc.tensor.transpose(tp4[:, i * 64:(i + 1) * 64],
                                    relu2[:, i * 128:(i + 1) * 128], idb[0:64, 0:64])
            r2T = stat.tile([128, 256], bf16, tag="r2T", name="r2T")
            nc.vector.tensor_copy(r2T[:], tp4[:])

            pc = pmm.tile([R, 1024], f32, tag="pmm", name="pmm")
            preload(pc[:, 0:256], btile("b_a2o"))
            for k in range(4):
                mm(pc[:, 0:256], r2T[:, k * 64:(k + 1) * 64],
                   wtile("a2w2", k, slice(0, 256)), stop=(k == 3))
            cHat = scr.tile([R, MEM], f32, tag="cHat", name="cHat")
            nc.scalar.activation(cHat[:], pc[:, 0:256], AF.Tanh)

            # ---- g1/g2 hidden
            pgh = pmm.tile([R, 1024], f32, tag="pmm", name="pmm")
            for gi, gn in ((0, "g1w1"), (1, "g2w1")):
                sl = pgh[:, gi * 512:(gi + 1) * 512]
                preload(sl, btile("b_g1h" if gi == 0 else "b_g2h"))
                for k in range(10):
                    mm(sl, bothT(k), wtile(gn, k, slice(0, 512)), stop=(k == 9))
            rg = scr.tile([R, 1024], bf16, tag="rg", name="rg")
            nc.vector.tensor_scalar_max(rg[:], pgh[:], 0.0)
            tp5 = ptp.tile([128, 512], bf16, tag="tp", name="tp")
            for i in range(8):
                nc.tensor.transpose(tp5[:, i * 64:(i + 1) * 64],
                                    rg[:, i * 128:(i + 1) * 128], idb[0:64, 0:64])
            rgT = stat.tile([128, 512], bf16, tag="rgT", name="rgT")
            nc.vector.tensor_copy(rgT[:], tp5[:])

            # ---- gammas (tanh-trick, 0.5 baked into g?w2 + bias)
            pga = pmm.tile([R, 1024], f32, tag="pmm", name="pmm")
            preload(pga[:, 0:512], btile("b_gw2o"))
            for k in range(4):
                mm(pga[:, 0:256], rgT[:, k * 64:(k + 1) * 64],
                   wtile("g1w2", k, slice(0, 256)), stop=(k == 3))
            for k in range(4):
                mm(pga[:, 256:512], rgT[:, 256 + k * 64:256 + (k + 1) * 64],
                   wtile("g2w2", k, slice(0, 256)), stop=(k == 3))
            Tg = scr.tile([R, 512], f32, tag="Tg", name="Tg")
            nc.scalar.activation(Tg[:], pga[:, 0:512], AF.Tanh)
            Gam = scr.tile([R, 512], f32, tag="Gam", name="Gam")
            nc.vector.tensor_scalar(Gam[:], Tg[:], 0.5, 0.5, ALU.mult, ALU.add)

            # ---- mem update
            ma = scr.tile([R, MEM], f32, tag="ma", name="ma")
            nc.vector.tensor_tensor(ma[:], Gam[:, 0:256], Mem_o[:], ALU.mult)
            mb = scr.tile([R, MEM], f32, tag="mb", name="mb")
            nc.vector.tensor_tensor(mb[:], Gam[:, 256:512], cHat[:], ALU.mult)
            nc.vector.tensor_tensor(Mem_n[:], ma[:], mb[:], ALU.add)

            tp6 = ptp.tile([128, 128], f32, tag="tp", name="tp")
            for i in range(2):
                nc.tensor.transpose(tp6[:, i * 64:(i + 1) * 64],
                                    Mem_n[:, i * 128:(i + 1) * 128], idf[0:64, 0:64])
            nc.vector.tensor_copy(memT[:], tp6[:])

        # ================= head (on device) =================
        # logits z_m = (H_m/2) @ fw_m.T + fb_m  (0.5 folded into wf)
        AX = self.mybir.AxisListType.X
        zf = pmm.tile([R, 1024], f32, tag="pmm", name="zf")
        preload(zf[:, 0:384], btile("b_f"))
        mm(zf[:, 0:128], hT[:, 0:64], htile("wf_l", 0, slice(0, 128)))
        mm(zf[:, 0:128], hT[:, 64:128], htile("wf_l", 1, slice(0, 128)),
           stop=True)
        mm(zf[:, 128:256], hT[:, 128:192], htile("wf_a", 0, slice(0, 128)),
           stop=True)
        mm(zf[:, 256:384], hT[:, 192:256], htile("wf_v", 0, slice(0, 128)),
           stop=True)
        zsb = scr.tile([R, 384], f32, tag="zsb", name="zsb")
        nc.vector.tensor_copy(zsb[:], zf[:, 0:384])
        # per-core per-modality max -> [64, 3] -> [3, 1]
        zmax = scr.tile([R, 3], f32, tag="zmax", name="zmax")
        for m in range(3):
            nc.vector.tensor_reduce(zmax[:, m:m + 1],
                                    zsb[:, m * 128:(m + 1) * 128], AX, ALU.max)
        tpm = ptp.tile([128, 64], f32, tag="tp", name="tpm")
        nc.tensor.transpose(tpm[0:3, 0:64], zmax[0:64, 0:3], idf[0:64, 0:64])
        mxc = scr.tile([128, 1], f32, tag="mxc", name="mxc")
        nc.vector.tensor_reduce(mxc[0:3, 0:1], tpm[0:3, 0:64], AX, ALU.max)
        # global max via AllReduce(max), then broadcast to [64, 3]
        mx_in = dram.tile([3, 1], f32, tag="mx_in", name="mx_in")
        mx_out = dram.tile([3, 1], f32, tag="mx_out", name="mx_out")
        nc.gpsimd.dma_start(mx_in[:], mxc[0:3, 0:1])
        nc.gpsimd.collective_compute(
            "AllReduce", ALU.max, replica_groups=[list(range(NC))],
            ins=[mx_in[:].opt()], outs=[mx_out[:].opt()])
        mrow = scr.tile([1, 3], f32, tag="mrow", name="mrow")
        nc.sync.dma_start(mrow[:], mx_out[:])
        pb = ptp.tile([128, 64], f32, tag="tp", name="pb")
        nc.tensor.matmul(pb[0:64, 0:3], onesf[:], mrow[:], start=True,
                         stop=True, skip_group_check=True)
        nmax = scr.tile([R, 3], f32, tag="nmax", name="nmax")
        nc.vector.tensor_scalar(nmax[:], pb[0:64, 0:3], -1.0, None, ALU.mult)
        # p = exp(z - max); u = z - max; lav = sum_m p_m * u_m
        Ex = scr.tile([R, 384], f32, tag="Ex", name="Ex")
        uu = scr.tile([R, 384], f32, tag="uu", name="uu")
        for m in range(3):
            sl = slice(m * 128, (m + 1) * 128)
            nc.scalar.activation(Ex[:, sl], zsb[:, sl], AF.Exp,
                                 bias=nmax[:, m:m + 1])
            nc.vector.tensor_scalar(uu[:, sl], zsb[:, sl], nmax[:, m:m + 1],
                                    None, ALU.add)
        pu = scr.tile([R, 384], f32, tag="pu", name="pu")
        nc.vector.tensor_tensor(pu[:], Ex[:], uu[:], ALU.mult)
        lv1 = scr.tile([R, 128], f32, tag="lv1", name="lv1")
        nc.vector.tensor_tensor(lv1[:], pu[:, 0:128], pu[:, 128:256], ALU.add)
        lav = scr.tile([R, 128], bf16, tag="lav", name="lav")
        nc.vector.tensor_tensor(lav[:], lv1[:], pu[:, 256:384], ALU.add)
        tpl = ptp.tile([128, 64], bf16, tag="tp", name="tpl")
        nc.tensor.transpose(tpl[:, 0:64], lav[:, 0:128], idb[0:64, 0:64])
        lavT = stat.tile([128, 64], bf16, tag="lavT", name="lavT")
        nc.vector.tensor_copy(lavT[:], tpl[:])
        # o1 = relu([lav | mem] @ o_w1.T + o_b1)   [64, 256]
        po = pmm.tile([R, 1024], f32, tag="pmm", name="po")
        preload(po[:, 0:256], btile("b_o1"))
        mm(po[:, 0:256], lavT[:, 0:64], htile("wo1", 0, slice(0, 256)))
        mm(po[:, 0:256], memT[:, 0:64], htile("wo1", 1, slice(0, 256)))
        mm(po[:, 0:256], memT[:, 64:128], htile("wo1", 2, slice(0, 256)),
           stop=True)
        o1r = scr.tile([R, 256], bf16, tag="o1r", name="o1r")
        nc.vector.tensor_scalar_max(o1r[:], po[:, 0:256], 0.0)
        tpo = ptp.tile([128, 128], bf16, tag="tp", name="tpo")
        for i in range(2):
            nc.tensor.transpose(tpo[:, i * 64:(i + 1) * 64],
                                o1r[:, i * 128:(i + 1) * 128], idb[0:64, 0:64])
        o1T = stat.tile([128, 128], bf16, tag="o1T", name="o1T")
        nc.vector.tensor_copy(o1T[:], tpo[:])
        poz = pmm.tile([R, 1024], f32, tag="pmm", name="poz")
        preload(poz[:, 0:1], btile("b_o2"))
        for i in range(2):
            mm(poz[:, 0:1], o1T[:, i * 64:(i + 1) * 64],
               htile("wo2", i, slice(0, 1)), stop=(i == 1))
        outsb = scr.tile([R, 1], f32, tag="outsb", name="outsb")
        nc.vector.tensor_copy(outsb[:], poz[:, 0:1])
        nc.sync.dma_start(out_d[:], outsb[:])
        if dbg_d is not None:
            # rows 0:64: [zsb 0:384 | nmax 384:387 | lav(f32) 512:640]
            # rows 0:128: [hT 640:896 | memT 896:1024] (bf16 upcast)
            nc.sync.dma_start(dbg_d[0:64, 0:384], zsb[:])
            nc.sync.dma_start(dbg_d[0:64, 384:387], nmax[:])
            lavf = scr.tile([R, 128], f32, tag="lavf", name="lavf")
            nc.vector.tensor_copy(lavf[:], lav[:])
            nc.sync.dma_start(dbg_d[0:64, 512:640], lavf[:])
            hTf = scr.tile([128, 384], f32, tag="hTf", name="hTf")
            nc.vector.tensor_copy(hTf[:, 0:256], hT[:])
            nc.vector.tensor_copy(hTf[:, 256:384], memT[:])
            nc.sync.dma_start(dbg_d[:, 640:1024], hTf[:])
            o1f = scr.tile([R, 256], f32, tag="o1f", name="o1f")
            nc.vector.tensor_copy(o1f[:], o1r[:])
            nc.sync.dma_start(dbg_d[64:128, 0:256], o1f[:])
            nc.sync.dma_start(dbgh_d[:], hsb[:])
            nc.sync.dma_start(dbgb_d[:], bsb[:])


_CACHED = {}


def _build_executor(nc):
    """Build (once) a cached jit'd shard_map executable mirroring
    bass2jax.run_bass_via_pjrt, so repeat calls skip retrace/relower."""
    import jax
    from jax.sharding import Mesh, PartitionSpec
    from jax.experimental.shard_map import shard_map
    from concourse import mybir
    from concourse.bass2jax import (_bass_exec_p, install_neuronx_cc_hook,
                                    partition_id_tensor)

    install_neuronx_cc_hook()

    partition_name = nc.partition_id_tensor.name if nc.partition_id_tensor else None
    in_names, out_names, out_avals = [], [], []
    for alloc in nc.m.functions[0].allocations:
        if not isinstance(alloc, mybir.MemoryLocationSet):
            continue
        name = alloc.memorylocations[0].name
        if alloc.kind == "ExternalInput":
            if name != partition_name:
                in_names.append(name)
        elif alloc.kind == "ExternalOutput":
            out_names.append(name)
            out_avals.append(jax.core.ShapedArray(
                tuple(alloc.tensor_shape), mybir.dt.np(alloc.dtype)))
    n_params = len(in_names)
    n_outs = len(out_names)
    all_in_names = in_names + out_names + ([partition_name] if partition_name else [])

    def _body(*args):
        operands = list(args)
        if partition_name is not None:
            operands.append(partition_id_tensor())
        outs = _bass_exec_p.bind(
            *operands,
            out_avals=tuple(out_avals),
            in_names=tuple(all_in_names),
            out_names=tuple(out_names),
            lowering_input_output_aliases=(),
            sim_require_finite=True,
            sim_require_nnan=True,
            nc=nc,
        )
        return tuple(outs)

    devices = jax.devices()[:NC]
    mesh = Mesh(np.asarray(devices), ("core",))
    in_specs = (PartitionSpec("core"),) * (n_params + n_outs)
    out_specs = (PartitionSpec("core"),) * n_outs
    donate = tuple(range(n_params, n_params + n_outs))
    sharded = jax.jit(
        shard_map(_body, mesh=mesh, in_specs=in_specs, out_specs=out_specs,
                  check_rep=False),
        donate_argnums=donate, keep_unused=True)

    def execute(arrs):
        """arrs: dict name -> global [NC*d0, ...] array (sharded on axis 0)."""
        concat_in = [arrs[n] for n in in_names]
        zeros = [np.zeros((NC * a.shape[0], *a.shape[1:]), a.dtype)
                 for a in out_avals]
        outs = sharded(*concat_in, *zeros)
        host = [np.asarray(o) for o in outs]
        return [
            {name: host[i].reshape(NC, *out_avals[i].shape)[c]
             for i, name in enumerate(out_names)}
            for c in range(NC)
        ]

    return execute


def _get_executor():
    if "exec" not in _CACHED:
        _CACHED["nc"] = build_nc()
        _CACHED["exec"] = _build_executor(_CACHED["nc"])
    return _CACHED["exec"]


def _postprocess(res, inputs):
    """The head runs on device; just stitch the per-core outputs."""
    return np.concatenate(
        [res[c]["o_out"] for c in range(NC)], axis=0).flatten().astype(np.float32)


def kernel(**inputs):
    x = np.asarray(inputs["x"])
    w8, whb = _prep_params(inputs)
    xall = _prep_x(x)
    execute = _get_executor()
    res = execute({"xt8": xall, "wsh": w8, "whb": whb})
    return _postprocess(res, inputs)
